# revision 87
# baseline (speedup 1.0000x reference)
"""MoE layer (GShard top-2 routing + per-expert FFN) on 8 Trainium2 NeuronCores.

Strategy (expert parallelism, ReduceScatter combine):
  - Router matmul (fp32, exact) is token-sharded: each core computes logits for
    its 1024-token shard, then an AllGather shares per-token routing scalars
    (idx1, idx2, g1, g2) with all cores.
  - Every core replicates the (cheap) global slot-assignment math: per-expert
    inclusive scans along the free dim + a triangular-matmul partition prefix
    give each token its capacity slot exactly as the reference's cumsum does.
  - Each core owns ONE expert. The slot->token map is built with local_scatter
    (per-partition scatter of token ids by slot), merged across partitions
    with a gpsimd partition all-reduce (each slot column has one writer), and
    read out column-major via a diagonal extraction (first 4 columns early so
    cb0's dispatch gathers start while the rest extract).
  - Dispatch: 16 indirect row gathers from x (bf16) + PE transposes give the
    [d, slot] layout; FFN in bf16 with fp32 accumulation:
    hT = gelu(w_gate^T @ dispT), eo = hT^T @ w_down (row-major out).
  - Combine via ReduceScatter: a [T+1,4] payload table (f1,f2,g1,g2 per token)
    is written to DRAM and gathered by the slot->token map, giving each slot
    its owner's gate. mm2's PSUM->SBUF copy scales eo rows by that gate, and
    the scaled rows are indirect-scattered into a [T+1,D] bf16 token-space
    partial buffer ("part", zero-filled on device early in the run; the
    collective verifier forbids IO tensors, so it must stay internal). A
    single bf16 ReduceScatter(add) over part[0:T] then yields each core's
    final output shard directly (tokens are shard-ordered), bounced to y
    (bf16) and cast to fp32 on the host.

  Scheduling notes (the TimelineSim cost model serializes all DMA on one
  device, FIFO by acquire time, and Tile schedules by dependency, not
  program order):
  - Big loads (weights, zero-fill) run as single-in-flight chains: each link
    is gated on the previous via a tiny DVE copy (weights) or a RAW
    self-copy (zero chunks), so routing-critical DMAs (payload write, AG
    rereads, dispatch gathers) never wait more than one ~3-6us link.
  - Indirect scatters claim a strided static window (rows 0,64,...,8128) of
    "part": cost is charged on the static AP (256KB, not 16.8MB), while the
    window still overlaps every zero chunk so Tile orders all scatters after
    the zero fill. Do NOT "slice" SBUF partition dims via rearrange in DMA
    APs (e.g. "(r p16) i -> r p16 i") — partition_size silently becomes r
    and the transfer writes garbage on hardware.
"""

import sys

if "/opt/trn_rl_repo" not in sys.path:
    sys.path.insert(0, "/opt/trn_rl_repo")

import numpy as np
import ml_dtypes

import concourse.bacc as bacc
import concourse.mybir as mybir
import concourse.tile as tile
from concourse import bass
from concourse import bass_isa
from concourse.bass_utils import run_bass_kernel_spmd

BF16 = mybir.dt.bfloat16
F32 = mybir.dt.float32
I16 = mybir.dt.int16
I32 = mybir.dt.int32
AF = mybir.ActivationFunctionType
OP = mybir.AluOpType
AX = mybir.AxisListType

B, S, D, E, F = 4, 2048, 1024, 8, 4096
T = B * S            # 8192 tokens
C = 2 * T // E       # 2048 capacity
NC = 8               # cores
SH = T // NC         # 1024 tokens per shard
CBLK = 512           # FFN slot-block
NCB = C // CBLK      # 4 blocks

LAST_RESULT = None   # BassKernelResults of the most recent run (for profiling)


def _build_program():
    nc = bacc.Bacc("TRN2", target_bir_lowering=False, debug=False, num_devices=NC)

    # ---- per-core external inputs ----
    xT_sh = nc.dram_tensor("xT_sh", [D, SH], F32, kind="ExternalInput").ap()
    xb = nc.dram_tensor("xb", [T + 1, D], BF16, kind="ExternalInput").ap()
    wg_d = nc.dram_tensor("wg", [D, E], F32, kind="ExternalInput").ap()
    wgt_d = nc.dram_tensor("wgt", [D, F], BF16, kind="ExternalInput").ap()
    wdn_d = nc.dram_tensor("wdn", [F, D], BF16, kind="ExternalInput").ap()
    cid_d = nc.dram_tensor("cid", [128, 1], F32, kind="ExternalInput").ap()
    slotid_d = nc.dram_tensor("slotid", [128, C // 128], F32, kind="ExternalInput").ap()
    # host-generated constants (gpsimd iota/affine_select aren't available)
    ident_d = nc.dram_tensor("ident", [128, 128], F32, kind="ExternalInput").ap()
    slmat_d = nc.dram_tensor("slmat", [128, 128], F32, kind="ExternalInput").ap()
    tidx_d = nc.dram_tensor("tidx", [128, 64], F32, kind="ExternalInput").ap()
    eidx_d = nc.dram_tensor("eidx", [128, E], F32, kind="ExternalInput").ap()
    y_d = nc.dram_tensor("y", [SH, D], BF16, kind="ExternalOutput").ap()
    # token-space partial output; zero-filled on device early in the run
    # (collectives may not read IO tensors, so this must stay internal)
    part_d = nc.dram_tensor("part", [T + 1, D], BF16).ap()

    zsrc_d = nc.dram_tensor("zsrc", [SH, D], BF16, kind="ExternalInput").ap()

    # ---- internal DRAM ----
    pay_in = nc.dram_tensor("pay_in", [4 * SH], F32).ap()
    pay_all = nc.dram_tensor("pay_all", [NC * 4 * SH], F32, addr_space="Shared").ap()
    pay_tab = nc.dram_tensor("pay_tab", [T + 1, 4], F32).ap()
    rs_out = nc.dram_tensor("rs_out", [SH, D], BF16).ap()

    with tile.TileContext(nc) as tc:
        with (
            tc.tile_pool(name="persist", bufs=1) as pp,
            tc.tile_pool(name="psum_s", bufs=2, space="PSUM") as pss,
        ):
            # route pool is opened here (before the persist consts, so xT's
            # DMA is emitted first) and closed explicitly before the FFN to
            # free its SBUF
            _route_cm = tc.tile_pool(name="route", bufs=1)
            pr = _route_cm.__enter__()

            # xT is the head of the critical path: emit it before everything
            # else so it gets the first DMA slot
            xT_sb = pr.tile([128, D // 128, SH], F32)
            nc.sync.dma_start(xT_sb[:], xT_sh.rearrange("(o q) t -> q o t", q=128))
            wg_sb = pr.tile([128, D // 128, E], F32)
            nc.sync.dma_start(wg_sb[:], wg_d.rearrange("(o q) e -> q o e", q=128))

            # zero-fill the token-space partial buffer. The 1MB chunks chain
            # off each other (RAW on the previous chunk), so at most one is in
            # flight and later critical DMAs (payload write, AG rereads,
            # dispatch gathers) wait at most ~3us for the DMA engines. Must
            # complete before the first eo scatter (~150us).
            ZC = 512
            nc.scalar.dma_start(part_d[0:ZC, :], zsrc_d[0:ZC, :])
            for zc in range(1, T // ZC):
                nc.scalar.dma_start(
                    part_d[ZC * zc : ZC * (zc + 1), :],
                    part_d[ZC * (zc - 1) : ZC * zc, :],
                )

            ident = pp.tile([128, 128], F32)
            nc.sync.dma_start(ident[:], ident_d[:])
            ident_bf = pp.tile([128, 128], BF16)
            nc.vector.tensor_copy(ident_bf[:], ident[:])
            cid = pp.tile([128, 1], F32)
            nc.sync.dma_start(cid[:], cid_d[:])
            slotid = pp.tile([128, C // 128], F32)
            nc.sync.dma_start(slotid[:], slotid_d[:])
            zeros64 = pp.tile([128, 64], F32)
            nc.vector.memset(zeros64[:], 0.0)
            ones128 = pp.tile([128, 128], F32)
            nc.vector.memset(ones128[:], 1.0)

            # resident expert weights (bf16)
            wgt_sb = pp.tile([128, D // 128, F], BF16)
            wdn_sb = pp.tile([128, F // 128, D], BF16)



            # persistent routing products
            tokc = pp.tile([128, C // 128], I32)    # dispatch: slot->token, col-major
            slotg = pp.tile([128, C // 128], F32)   # gate per slot, col-major

            # =============== ROUTER (token shard, fp32) ===============
            if True:
                sl = pr.tile([128, 128], F32)
                nc.sync.dma_start(sl[:], slmat_d[:])
                tif = pr.tile([128, 64], F32)
                nc.sync.dma_start(tif[:], tidx_d[:])
                eidx = pr.tile([128, E], F32)
                nc.sync.dma_start(eidx[:], eidx_d[:])



                lg = pr.tile([128, 8, E], F32)  # logits, token pos j = 128*tt + p
                for tt in range(8):
                    ps = pss.tile([128, E], F32, space="PSUM", tag="ps_small")
                    for kd in range(8):
                        nc.tensor.matmul(
                            ps[:],
                            lhsT=xT_sb[:, kd, 128 * tt : 128 * tt + 128],
                            rhs=wg_sb[:, kd, :],
                            start=(kd == 0),
                            stop=(kd == 7),
                        )
                    nc.vector.tensor_copy(lg[:, tt, :], ps[:])

                m1x = pr.tile([128, 8], F32)
                nc.vector.tensor_reduce(m1x[:], lg[:], AX.X, OP.max)


                is1 = pr.tile([128, 8, E], F32)
                nc.vector.tensor_tensor(
                    out=is1[:], in0=lg[:], in1=m1x[:, :, None].to_broadcast([128, 8, E]),
                    op=OP.is_equal,
                )
                l2 = pr.tile([128, 8, E], F32)
                nc.vector.scalar_tensor_tensor(
                    out=l2[:], in0=is1[:], scalar=-1e30, in1=lg[:], op0=OP.mult, op1=OP.add,
                )
                m2x = pr.tile([128, 8], F32)
                nc.vector.tensor_reduce(m2x[:], l2[:], AX.X, OP.max)
                is2 = pr.tile([128, 8, E], F32)
                nc.vector.tensor_tensor(
                    out=is2[:], in0=l2[:], in1=m2x[:, :, None].to_broadcast([128, 8, E]),
                    op=OP.is_equal,
                )

                # argmax index = sum(mask * eidx) along E
                i1f = pr.tile([128, 8], F32)
                sc1a = pr.tile([128, 8, E], F32, tag="am_scr_a")
                nc.vector.tensor_tensor(
                    out=sc1a[:], in0=is1[:], in1=eidx[:, None, :].to_broadcast([128, 8, E]),
                    op=OP.mult,
                )
                nc.vector.tensor_reduce(i1f[:], sc1a[:], AX.X, OP.add)
                i2f = pr.tile([128, 8], F32)
                sc2a = pr.tile([128, 8, E], F32, tag="am_scr_b")
                nc.vector.tensor_tensor(
                    out=sc2a[:], in0=is2[:], in1=eidx[:, None, :].to_broadcast([128, 8, E]),
                    op=OP.mult,
                )
                nc.vector.tensor_reduce(i2f[:], sc2a[:], AX.X, OP.add)

                # top-2 softmax gates: g1 = 1/(1+exp(m2-m1)), g2 = 1-g1
                dm = pr.tile([128, 8], F32)
                nc.vector.tensor_tensor(out=dm[:], in0=m2x[:], in1=m1x[:], op=OP.subtract)
                e2 = pr.tile([128, 8], F32)
                nc.scalar.activation(e2[:], dm[:], AF.Exp)
                den = pr.tile([128, 8], F32)
                nc.vector.tensor_scalar_add(den[:], e2[:], 1.0)
                g1 = pr.tile([128, 8], F32)
                nc.vector.reciprocal(g1[:], den[:])
                g2 = pr.tile([128, 8], F32)
                nc.vector.tensor_tensor(out=g2[:], in0=e2[:], in1=g1[:], op=OP.mult)

                pk = pr.tile([128, 4, 8], F32)
                nc.vector.tensor_copy(pk[:, 0, :], i1f[:])
                nc.vector.tensor_copy(pk[:, 1, :], i2f[:])
                nc.vector.tensor_copy(pk[:, 2, :], g1[:])
                nc.vector.tensor_copy(pk[:, 3, :], g2[:])
                nc.sync.dma_start(pay_in.rearrange("(a p tt) -> p a tt", a=4, p=128), pk[:])

                # gate_proj weights: 2MB links, each gated on the previous via
                # a tiny DVE copy; the chain head hangs off the payload pack so
                # the stream starts right after the (critical) payload write
                # and never monopolizes the DMA engines
                wgt_v = wgt_d.rearrange("(o q) f -> q o f", q=128)
                for h in range(4):
                    src = pk[0:1, 3, 0:2] if h == 0 else wgt_sb[0:1, 2 * (h - 1), 0:2]
                    nc.vector.tensor_copy(wgt_sb[0:1, 2 * h, 0:2], src)
                    nc.sync.dma_start(
                        wgt_sb[:, 2 * h : 2 * (h + 1), :], wgt_v[:, 2 * h : 2 * (h + 1), :]
                    )

                nc.gpsimd.collective_compute(
                    "AllGather", OP.bypass,
                    replica_groups=[list(range(NC))],
                    ins=[pay_in[:].opt()], outs=[pay_all[:].opt()],
                )

                nc.scalar.dma_start(pay_tab[T : T + 1, :], zeros64[0:1, 0:4])

                # reread all 4 arrays into global routing layout [128, 64]
                # (t = 64p + i). NOTE: a merged/rearranged form is NOT safe
                # here — splitting the SBUF partition dim (e.g. "(r p16) i ->
                # r p16 i") silently drops partition semantics (partition_size
                # becomes r) and writes garbage on hardware.
                rt = pr.tile([128, 4, 64], F32)
                pay_view = pay_all.rearrange("(r a p16 i) -> r p16 a i", r=NC, a=4, p16=16)
                for r in range(NC):
                    nc.sync.dma_start(rt[16 * r : 16 * r + 16, :, :], pay_view[r])
                i1r, i2r = rt[:, 0, :], rt[:, 1, :]
                g1r, g2r = rt[:, 2, :], rt[:, 3, :]



                # =============== SLOT ASSIGNMENT (replicated) ===============
                # (the real compiler only allows generic vector ops on DVE,
                # so both choice chains share it)
                v1, v2 = nc.vector, nc.vector
                m1 = pr.tile([128, E, 64], F32)
                m2 = pr.tile([128, E, 64], F32)
                sc1 = pr.tile([128, E, 64], F32)
                sc2 = pr.tile([128, E, 64], F32)
                v1.tensor_tensor(
                    out=m1[:], in0=i1r[:, None, :].to_broadcast([128, E, 64]),
                    in1=eidx[:, :, None].to_broadcast([128, E, 64]), op=OP.is_equal,
                )
                v2.tensor_tensor(
                    out=m2[:], in0=i2r[:, None, :].to_broadcast([128, E, 64]),
                    in1=eidx[:, :, None].to_broadcast([128, E, 64]), op=OP.is_equal,
                )
                for e in range(E):
                    v1.tensor_tensor_scan(sc1[:, e, :], m1[:, e, :], zeros64[:], 0.0, op0=OP.add, op1=OP.add)
                    v2.tensor_tensor_scan(sc2[:, e, :], m2[:, e, :], zeros64[:], 0.0, op0=OP.add, op1=OP.add)
                tot1 = pr.tile([128, E], F32)
                tot2 = pr.tile([128, E], F32)
                v1.tensor_copy(tot1[:], sc1[:, :, 63])
                v2.tensor_copy(tot2[:], sc2[:, :, 63])

                of1_ps = pss.tile([128, E], F32, space="PSUM", tag="ps_small")
                nc.tensor.matmul(of1_ps[:], lhsT=sl[:], rhs=tot1[:], start=True, stop=True)
                of1 = pr.tile([128, E], F32)
                nc.vector.tensor_scalar_add(of1[:], of1_ps[:], -1.0)
                of2_ps = pss.tile([128, E], F32, space="PSUM", tag="ps_small")
                nc.tensor.matmul(of2_ps[:], lhsT=sl[:], rhs=tot2[:], start=True, stop=False)
                nc.tensor.matmul(of2_ps[:], lhsT=ones128[:], rhs=tot1[:], start=False, stop=True)
                of2 = pr.tile([128, E], F32)
                nc.vector.tensor_scalar_add(of2[:], of2_ps[:], -1.0)

                def loc_s(vv, sc, m, of, tag):
                    tmp = pr.tile([128, E, 64], F32, tag=f"loc_tmp{tag}")
                    for e in range(E):
                        vv.scalar_tensor_tensor(
                            out=tmp[:, e, :], in0=sc[:, e, :], scalar=of[:, e : e + 1],
                            in1=m[:, e, :], op0=OP.add, op1=OP.mult,
                        )
                    cur, w = tmp, E
                    while w > 1:
                        nxt = pr.tile([128, w // 2, 64], F32, tag=f"loc_s{tag}{w}")
                        vv.tensor_tensor(out=nxt[:], in0=cur[:, : w // 2, :], in1=cur[:, w // 2 :, :], op=OP.add)
                        cur, w = nxt, w // 2
                    return cur  # [128, 1, 64]

                l1s = loc_s(v1, sc1, m1, of1, "a")[:, 0, :]
                l2s = loc_s(v2, sc2, m2, of2, "b")[:, 0, :]

                def keep_f(vv, ls, ir, tag):
                    kp = pr.tile([128, 64], F32, tag=f"kp{tag}")
                    vv.tensor_scalar(out=kp[:], in0=ls, scalar1=float(C), scalar2=None, op0=OP.is_lt)
                    lc = pr.tile([128, 64], F32, tag=f"lc{tag}")
                    vv.tensor_scalar(out=lc[:], in0=ls, scalar1=float(C - 1), scalar2=None, op0=OP.min)
                    f = pr.tile([128, 64], F32, tag=f"f{tag}")
                    vv.scalar_tensor_tensor(out=f[:], in0=ir, scalar=float(C), in1=lc[:], op0=OP.mult, op1=OP.add)
                    return f, kp

                f1, kp1 = keep_f(v1, l1s, i1r, "a")
                f2, kp2 = keep_f(v2, l2s, i2r, "b")

                # payload table rows t = 64p + i: (f1, f2, g1, g2)
                pt_sb = pr.tile([128, 64, 4], F32)
                nc.vector.tensor_copy(pt_sb[:, :, 0], f1[:])
                nc.vector.tensor_copy(pt_sb[:, :, 1], f2[:])
                nc.vector.tensor_copy(pt_sb[:, :, 2], g1r)
                nc.vector.tensor_copy(pt_sb[:, :, 3], g2r)
                nc.sync.dma_start(
                    pay_tab[0:T, :].rearrange("(p i) c -> p i c", p=128), pt_sb[:]
                )

                # ====== SLOT -> TOKEN MAP (local_scatter + merge + diagonal) ======
                tp1 = pr.tile([128, 64], F32)
                nc.vector.tensor_scalar_add(tp1[:], tif[:], 1.0)   # token id + 1

                def slot_halves(vv, ls, ir, kp, tag):
                    # sel = (expert == cid) && kept; slot+1 where selected else 0
                    isc = pr.tile([128, 64], F32, tag=f"isc{tag}")
                    vv.tensor_tensor(out=isc[:], in0=ir, in1=cid[:, 0:1].to_broadcast([128, 64]), op=OP.is_equal)
                    sel = pr.tile([128, 64], F32, tag=f"sel{tag}")
                    vv.tensor_tensor(out=sel[:], in0=isc[:], in1=kp[:], op=OP.mult)
                    sp1 = pr.tile([128, 64], F32, tag=f"sp1{tag}")  # sel ? slot+1 : 0
                    vv.tensor_scalar_add(sp1[:], ls, 1.0)
                    vv.tensor_tensor(out=sp1[:], in0=sp1[:], in1=sel[:], op=OP.mult)
                    # lo half: slot in [0, 1024): idx = slot, else -1
                    mlo = pr.tile([128, 64], F32, tag=f"mlo{tag}")
                    vv.tensor_scalar(out=mlo[:], in0=sp1[:], scalar1=1024.0, scalar2=None, op0=OP.is_le)
                    vv.tensor_tensor(out=mlo[:], in0=mlo[:], in1=sel[:], op=OP.mult)
                    ilo = pr.tile([128, 64], F32, tag=f"ilo{tag}")
                    vv.tensor_tensor(out=ilo[:], in0=mlo[:], in1=sp1[:], op=OP.mult)
                    vv.tensor_scalar_add(ilo[:], ilo[:], -1.0)
                    # hi half: slot in [1024, 2048): idx = slot - 1024, else -1
                    mhi = pr.tile([128, 64], F32, tag=f"mhi{tag}")
                    vv.tensor_scalar(out=mhi[:], in0=sp1[:], scalar1=1024.0, scalar2=None, op0=OP.is_gt)
                    ihi = pr.tile([128, 64], F32, tag=f"ihi{tag}")
                    vv.tensor_scalar_add(ihi[:], sp1[:], -1024.0)
                    vv.tensor_tensor(out=ihi[:], in0=ihi[:], in1=mhi[:], op=OP.mult)
                    vv.tensor_scalar_add(ihi[:], ihi[:], -1.0)
                    return ilo, ihi

                i1lo, i1hi = slot_halves(v1, l1s, i1r, kp1, "a")
                i2lo, i2hi = slot_halves(v2, l2s, i2r, kp2, "b")

                data128 = pr.tile([128, 128], I16)
                v1.tensor_copy(data128[:, :64], tp1[:])
                v2.tensor_copy(data128[:, 64:], tp1[:])
                idxlo = pr.tile([128, 128], I16)
                v1.tensor_copy(idxlo[:, :64], i1lo[:])
                v2.tensor_copy(idxlo[:, 64:], i2lo[:])
                idxhi = pr.tile([128, 128], I16)
                v1.tensor_copy(idxhi[:, :64], i1hi[:])
                v2.tensor_copy(idxhi[:, 64:], i2hi[:])

                dst_lo = pr.tile([128, 1024], I16)
                nc.gpsimd.local_scatter(dst_lo[:], data128[:], idxlo[:], channels=128, num_elems=1024, num_idxs=128)
                dst_hi = pr.tile([128, 1024], I16)
                nc.gpsimd.local_scatter(dst_hi[:], data128[:], idxhi[:], channels=128, num_elems=1024, num_idxs=128)

                # merge across partitions: each slot column has at most one
                # nonzero writer, so a gpsimd partition all-reduce (max)
                # replicates the slot->token map onto every partition
                merged = pr.tile([128, 2, 1024], F32)  # map+1 on all partitions
                nc.gpsimd.partition_all_reduce(
                    merged[:, 0, :], dst_lo[:], channels=128, reduce_op=bass_isa.ReduceOp.max
                )
                nc.gpsimd.partition_all_reduce(
                    merged[:, 1, :], dst_hi[:], channels=128, reduce_op=bass_isa.ReduceOp.max
                )

                # diagonal extraction: tokraw[p, k] = merged-flat[128k + p]
                tokraw = pr.tile([128, C // 128], F32)
                scratch = pr.tile([128, 128], F32, tag="diag_scr")
                mview = merged[:].rearrange("p a b -> p (a b)")
                scratch2 = pr.tile([128, 128], F32, tag="diag_scr2")
                iszero = pr.tile([128, C // 128], F32)

                def diag_cols(k0, k1):
                    # extract columns [k0,k1), sanitize (0 -> T+1; v -> v-1),
                    # and publish them to tokc so dependent gathers can start
                    for k in range(k0, k1):
                        vv, scr = (v1, scratch) if k % 2 == 0 else (v2, scratch2)
                        vv.scalar_tensor_tensor(
                            out=scr[:], in0=mview[:, 128 * k : 128 * (k + 1)], scalar=0.0,
                            in1=ident[:], op0=OP.add, op1=OP.mult,
                            accum_out=tokraw[:, k : k + 1],
                        )
                    nc.vector.tensor_scalar(out=iszero[:, k0:k1], in0=tokraw[:, k0:k1], scalar1=0.0, scalar2=None, op0=OP.is_equal)
                    nc.vector.scalar_tensor_tensor(
                        out=tokraw[:, k0:k1], in0=iszero[:, k0:k1], scalar=float(T + 1),
                        in1=tokraw[:, k0:k1], op0=OP.mult, op1=OP.add,
                    )
                    nc.vector.tensor_scalar_add(tokraw[:, k0:k1], tokraw[:, k0:k1], -1.0)
                    nc.vector.tensor_copy(tokc[:, k0:k1], tokraw[:, k0:k1])

                # cb0's dispatch gathers need only the first 4 columns: emit
                # them first so the FFN pipeline starts while the rest extract
                diag_cols(0, CBLK // 128)
                diag_cols(CBLK // 128, C // 128)

                # down_proj weights: 2MB links chained like wgt, head gated on
                # the slot map; needed only by cb0's mm2 (~140us)
                wdn_v = wdn_d.rearrange("(o q) d -> q o d", q=128)
                for h in range(4):
                    src = tokraw[0:1, 0:2] if h == 0 else wdn_sb[0:1, 8 * (h - 1), 0:2]
                    nc.vector.tensor_copy(wdn_sb[0:1, 8 * h, 0:2], src)
                    nc.sync.dma_start(
                        wdn_sb[:, 8 * h : 8 * (h + 1), :], wdn_v[:, 8 * h : 8 * (h + 1), :]
                    )

            _route_cm.__exit__(None, None, None)

            # =============== EXPERT FFN (bf16) ===============
            with (
                tc.tile_pool(name="ffn", bufs=1) as pf,
                tc.tile_pool(name="ffn_db", bufs=2) as pfd,
                tc.tile_pool(name="ffn_dr", bufs=4) as pdr,
                tc.tile_pool(name="psum_mm", bufs=2, space="PSUM") as psm,
            ):
                def emit_dispatch(cb):
                    # gather 4 x 128 slot rows; each row block is transposed
                    # into dispT by ONE xbar DMA transpose (64 tiles x 14ns)
                    # instead of 8 PE transposes + 8 DVE copies — the work
                    # moves to the DMA device, which is idle during the FFN
                    KT = CBLK // 128
                    dispT = pfd.tile([128, D // 128, CBLK], BF16, tag="dispT")
                    for kt in range(KT):
                        k = KT * cb + kt
                        drow = pdr.tile([128, D], BF16, tag="drow")
                        nc.gpsimd.indirect_dma_start(
                            out=drow[:], out_offset=None, in_=xb[:],
                            in_offset=bass.IndirectOffsetOnAxis(ap=tokc[:, k : k + 1], axis=0),
                        )
                        nc.sync.dma_start_transpose(
                            dispT[:, :, 128 * kt : 128 * (kt + 1)], drow[:]
                        )
                    return dispT

                next_dispT = emit_dispatch(0)
                for cb in range(NCB):
                    dispT = next_dispT

                    if cb == 0:
                        # slot gates: gather payload rows by slot owner, then
                        # gate = (f1==slot)*g1 + (f2==slot)*g2. Emitted after
                        # cb0's dispatch gathers so they don't delay the FFN
                        # start on the (in-order) gpsimd queue; results are
                        # only needed by cb0's mm2 scale, ~100us later.
                        pg = pf.tile([128, C // 128, 4], F32, tag="pg")
                        for k in range(C // 128):
                            nc.gpsimd.indirect_dma_start(
                                out=pg[:, k, :], out_offset=None, in_=pay_tab[:],
                                in_offset=bass.IndirectOffsetOnAxis(ap=tokc[:, k : k + 1], axis=0),
                            )
                        is1g = pf.tile([128, C // 128], F32, tag="is1g")
                        nc.vector.tensor_tensor(out=is1g[:], in0=pg[:, :, 0], in1=slotid[:], op=OP.is_equal)
                        is2g = pf.tile([128, C // 128], F32, tag="is2g")
                        nc.vector.tensor_tensor(out=is2g[:], in0=pg[:, :, 1], in1=slotid[:], op=OP.is_equal)
                        ga = pf.tile([128, C // 128], F32, tag="ga")
                        nc.vector.tensor_tensor(out=ga[:], in0=is1g[:], in1=pg[:, :, 2], op=OP.mult)
                        gb = pf.tile([128, C // 128], F32, tag="gb")
                        nc.vector.tensor_tensor(out=gb[:], in0=is2g[:], in1=pg[:, :, 3], op=OP.mult)
                        nc.vector.tensor_tensor(out=slotg[:], in0=ga[:], in1=gb[:], op=OP.add)

                    hT = pf.tile([128, F // 128, CBLK], BF16, tag="hT")
                    for ft in range(F // 128):
                        ps1 = psm.tile([128, CBLK], F32, space="PSUM", tag="ps1")
                        for kd in range(D // 128):
                            nc.tensor.matmul(
                                ps1[:],
                                lhsT=wgt_sb[:, kd, 128 * ft : 128 * ft + 128],
                                rhs=dispT[:, kd, :],
                                start=(kd == 0), stop=(kd == D // 128 - 1),
                            )
                        nc.scalar.activation(hT[:, ft, :], ps1[:], AF.Gelu)

                    # prefetch the next block's dispatch AFTER this block's mm1
                    # emission: Tile's in-order PE queue then runs those
                    # transposes only when their gathers are long done, instead
                    # of idling PE mid-mm1 waiting for them
                    if cb + 1 < NCB:
                        next_dispT = emit_dispatch(cb + 1)

                    # mm2 with swapped operands: eo[c, d] = hT.T @ w_down -> row-major out
                    # PSUM->SBUF copy applies the slot gate (per-partition scalar)
                    eo_sb = pf.tile([128, CBLK // 128, D], BF16, tag="eo_sb")
                    for ct in range(CBLK // 128):
                        k = (CBLK // 128) * cb + ct
                        for dc in range(D // 512):
                            ps2 = psm.tile([128, 512], F32, space="PSUM", tag="ps2")
                            for ft in range(F // 128):
                                nc.tensor.matmul(
                                    ps2[:],
                                    lhsT=hT[:, ft, 128 * ct : 128 * ct + 128],
                                    rhs=wdn_sb[:, ft, 512 * dc : 512 * dc + 512],
                                    start=(ft == 0), stop=(ft == F // 128 - 1),
                                )
                            nc.vector.tensor_scalar_mul(
                                eo_sb[:, ct, 512 * dc : 512 * dc + 512], ps2[:],
                                slotg[:, k : k + 1],
                            )
                        # scatter scaled rows into token space (trash row T for
                        # empty slots; their eo is exactly 0 anyway). The static
                        # out AP is a 128-row window (offset must be 0): the
                        # actual rows come from the dynamic offsets, but the
                        # cost model (and descriptor count) key on the static
                        # AP, which must not span the whole 16.8MB tensor.
                        nc.gpsimd.indirect_dma_start(
                            out=part_d[0:T, :].rearrange("(a b) d -> a (b d)", b=64)[:, 0:D],
                            out_offset=bass.IndirectOffsetOnAxis(ap=tokc[:, k : k + 1], axis=0),
                            in_=eo_sb[:, ct, :], in_offset=None,
                        )

            # =============== COMBINE: ReduceScatter over token space ===============
            # (collectives may not read or write IO tensors: internal in/out,
            # then a small DMA moves the reduced shard to y)
            nc.gpsimd.collective_compute(
                "ReduceScatter", OP.add,
                replica_groups=[list(range(NC))],
                ins=[part_d[0:T, :].opt()], outs=[rs_out[:].opt()],
            )
            nc.sync.dma_start(y_d[:], rs_out[:])

    nc.compile()
    return nc


_PROGRAM = None


def _get_program():
    global _PROGRAM
    if _PROGRAM is None:
        _PROGRAM = _build_program()
    return _PROGRAM


def host_constants():
    p = np.arange(128)
    return {
        "ident": np.eye(128, dtype=np.float32),
        "slmat": (np.arange(128)[None, :] > p[:, None]).astype(np.float32),
        "tidx": (64 * p[:, None] + np.arange(64)[None, :]).astype(np.float32),
        "eidx": np.tile(np.arange(E, dtype=np.float32), (128, 1)),
    }


def _make_in_maps(x, wg, w_gate, w_down):
    x = np.asarray(x, np.float32)
    wg_np = np.asarray(wg, np.float32)
    w_gate_np = np.asarray(w_gate, np.float32)
    w_down_np = np.asarray(w_down, np.float32)

    tokens = x.reshape(T, D)
    xb = np.zeros((T + 1, D), ml_dtypes.bfloat16)
    xb[:T] = tokens.astype(ml_dtypes.bfloat16)

    # shard m holds tokens [SH*m, SH*(m+1)); its xT columns are permuted so that
    # matmul tile position j = 128*tt + p corresponds to local token 8*p + tt,
    # making the routing payload DMA contiguous.
    j = np.arange(SH)
    perm = 8 * (j % 128) + j // 128  # local token index at column position j
    consts = host_constants()
    p = np.arange(128)
    kk = np.arange(C // 128)

    in_maps = []
    for m in range(NC):
        shard = tokens[SH * m : SH * (m + 1)]
        xT_sh = np.ascontiguousarray(shard[perm].T)
        in_maps.append({
            "xT_sh": xT_sh,
            "xb": xb,
            "wg": wg_np,
            "wgt": np.ascontiguousarray(w_gate_np[m].astype(ml_dtypes.bfloat16)),
            "wdn": np.ascontiguousarray(w_down_np[m].astype(ml_dtypes.bfloat16)),
            "cid": np.full((128, 1), float(m), np.float32),
            "slotid": (m * C + 128 * kk[None, :] + p[:, None]).astype(np.float32),
            "zsrc": np.zeros((SH, D), ml_dtypes.bfloat16),
            **consts,
        })
    return in_maps


def kernel(x, wg, w_gate, w_down, _trace=False):
    global LAST_RESULT
    x = np.asarray(x, np.float32)
    in_maps = _make_in_maps(x, wg, w_gate, w_down)

    nc = _get_program()
    res = run_bass_kernel_spmd(nc, in_maps, core_ids=list(range(NC)), trace=_trace)
    LAST_RESULT = res
    out = np.concatenate([res.results[m]["y"] for m in range(NC)], axis=0)
    return out.reshape(B, S, D).astype(x.dtype)


def bench(x, wg, w_gate, w_down, iters=6):
    """Measure per-execution wall time with device-resident inputs.

    Returns (output, per_call_seconds_list) where each call gets freshly
    zeroed (donated) output buffers, matching run_bass_via_pjrt semantics.
    """
    import time
    import jax
    from jax.sharding import Mesh, PartitionSpec, NamedSharding
    from jax.experimental.shard_map import shard_map
    import concourse.mybir as _mybir
    from concourse.bass2jax import _bass_exec_p, install_neuronx_cc_hook, partition_id_tensor

    install_neuronx_cc_hook()
    nc = _get_program()

    x = np.asarray(x, np.float32)
    in_maps = _make_in_maps(x, wg, w_gate, w_down)

    in_names, out_names, out_avals, zero_outs = [], [], [], []
    for alloc in nc.m.functions[0].allocations:
        if not isinstance(alloc, _mybir.MemoryLocationSet):
            continue
        name = alloc.memorylocations[0].name
        if alloc.kind == "ExternalInput":
            if nc.partition_id_tensor is None or name != nc.partition_id_tensor.name:
                in_names.append(name)
        elif alloc.kind == "ExternalOutput":
            shape = tuple(alloc.tensor_shape)
            dtype = _mybir.dt.np(alloc.dtype)
            out_names.append(name)
            out_avals.append(jax.core.ShapedArray(shape, dtype))
            zero_outs.append(np.zeros(shape, dtype))
    n_params = len(in_names)
    all_in_names = in_names + out_names
    if nc.partition_id_tensor is not None:
        all_in_names = all_in_names + [nc.partition_id_tensor.name]

    def _body(*args):
        operands = list(args)
        if nc.partition_id_tensor is not None:
            operands.append(partition_id_tensor())
        outs = _bass_exec_p.bind(
            *operands,
            out_avals=tuple(out_avals),
            in_names=tuple(all_in_names),
            out_names=tuple(out_names),
            lowering_input_output_aliases=(),
            sim_require_finite=True,
            sim_require_nnan=True,
            nc=nc,
        )
        return tuple(outs)

    devices = jax.devices()[:NC]
    mesh = Mesh(np.asarray(devices), ("core",))
    nsh = NamedSharding(mesh, PartitionSpec("core"))
    n_outs = len(out_avals)
    donate = tuple(range(n_params, n_params + n_outs))
    sharded = jax.jit(
        shard_map(_body, mesh=mesh, in_specs=(PartitionSpec("core"),) * (n_params + n_outs),
                  out_specs=(PartitionSpec("core"),) * n_outs, check_rep=False),
        donate_argnums=donate, keep_unused=True,
    )

    concat_in = [
        jax.device_put(np.concatenate([np.asarray(in_maps[c][nm]) for c in range(NC)], axis=0), nsh)
        for nm in in_names
    ]
    zero_sets = [
        [jax.device_put(np.zeros((NC * z.shape[0], *z.shape[1:]), z.dtype), nsh) for z in zero_outs]
        for _ in range(iters + 1)
    ]

    out = sharded(*concat_in, *zero_sets[0])  # warmup + compile
    jax.block_until_ready(out)
    times = []
    for it in range(iters):
        t0 = time.perf_counter()
        out = sharded(*concat_in, *zero_sets[it + 1])
        jax.block_until_ready(out)
        times.append(time.perf_counter() - t0)

    outs = {
        nm: np.asarray(out[i]).reshape(NC, *out_avals[i].shape) for i, nm in enumerate(out_names)
    }
    y = np.concatenate([outs["y"][m] for m in range(NC)], axis=0).reshape(B, S, D).astype(x.dtype)
    return y, times


# revision 88
# speedup vs baseline: 1.0234x; 1.0234x over previous
"""MoE layer (GShard top-2 routing + per-expert FFN) on 8 Trainium2 NeuronCores.

Strategy (expert parallelism, ReduceScatter combine):
  - Router matmul (fp32, exact) is token-sharded: each core computes logits for
    its 1024-token shard, then an AllGather shares per-token routing scalars
    (idx1, idx2, g1, g2) with all cores.
  - Every core replicates the (cheap) global slot-assignment math: per-expert
    inclusive scans along the free dim + a triangular-matmul partition prefix
    give each token its capacity slot exactly as the reference's cumsum does.
  - Each core owns ONE expert. The slot->token map is built with local_scatter
    (per-partition scatter of token ids by slot), merged across partitions
    with a gpsimd partition all-reduce (each slot column has one writer), and
    read out column-major via a diagonal extraction (first 4 columns early so
    cb0's dispatch gathers start while the rest extract).
  - Dispatch: 16 indirect row gathers from x (bf16) + PE transposes give the
    [d, slot] layout; FFN in bf16 with fp32 accumulation:
    hT = gelu(w_gate^T @ dispT), eo = hT^T @ w_down (row-major out).
  - Combine via ReduceScatter: a [T+1,4] payload table (f1,f2,g1,g2 per token)
    is written to DRAM and gathered by the slot->token map, giving each slot
    its owner's gate. mm2's PSUM->SBUF copy scales eo rows by that gate, and
    the scaled rows are indirect-scattered into a [T+1,D] bf16 token-space
    partial buffer ("part", zero-filled on device early in the run; the
    collective verifier forbids IO tensors, so it must stay internal). A
    single bf16 ReduceScatter(add) over part[0:T] then yields each core's
    final output shard directly (tokens are shard-ordered), bounced to y
    (bf16) and cast to fp32 on the host.

  Scheduling notes (the TimelineSim cost model serializes all DMA on one
  device, FIFO by acquire time, and Tile schedules by dependency, not
  program order):
  - Big loads (weights, zero-fill) run as single-in-flight chains: each link
    is gated on the previous via a tiny DVE copy (weights) or a RAW
    self-copy (zero chunks), so routing-critical DMAs (payload write, AG
    rereads, dispatch gathers) never wait more than one ~3-6us link.
  - Indirect scatters claim a strided static window (rows 0,64,...,8128) of
    "part": cost is charged on the static AP (256KB, not 16.8MB), while the
    window still overlaps every zero chunk so Tile orders all scatters after
    the zero fill. Do NOT "slice" SBUF partition dims via rearrange in DMA
    APs (e.g. "(r p16) i -> r p16 i") — partition_size silently becomes r
    and the transfer writes garbage on hardware.
"""

import sys

if "/opt/trn_rl_repo" not in sys.path:
    sys.path.insert(0, "/opt/trn_rl_repo")

import numpy as np
import ml_dtypes

import concourse.bacc as bacc
import concourse.mybir as mybir
import concourse.tile as tile
from concourse import bass
from concourse import bass_isa
from concourse.bass_utils import run_bass_kernel_spmd

BF16 = mybir.dt.bfloat16
F32 = mybir.dt.float32
I16 = mybir.dt.int16
I32 = mybir.dt.int32
AF = mybir.ActivationFunctionType
OP = mybir.AluOpType
AX = mybir.AxisListType

B, S, D, E, F = 4, 2048, 1024, 8, 4096
T = B * S            # 8192 tokens
C = 2 * T // E       # 2048 capacity
NC = 8               # cores
SH = T // NC         # 1024 tokens per shard
CBLK = 512           # FFN slot-block
NCB = C // CBLK      # 4 blocks

LAST_RESULT = None   # BassKernelResults of the most recent run (for profiling)


def _build_program():
    nc = bacc.Bacc("TRN2", target_bir_lowering=False, debug=False, num_devices=NC)

    # ---- per-core external inputs ----
    xT_sh = nc.dram_tensor("xT_sh", [D, SH], F32, kind="ExternalInput").ap()
    xb = nc.dram_tensor("xb", [T + 1, D], BF16, kind="ExternalInput").ap()
    wg_d = nc.dram_tensor("wg", [D, E], F32, kind="ExternalInput").ap()
    wgt_d = nc.dram_tensor("wgt", [D, F], BF16, kind="ExternalInput").ap()
    wdn_d = nc.dram_tensor("wdn", [F, D], BF16, kind="ExternalInput").ap()
    cid_d = nc.dram_tensor("cid", [128, 1], F32, kind="ExternalInput").ap()
    slotid_d = nc.dram_tensor("slotid", [128, C // 128], F32, kind="ExternalInput").ap()
    # host-generated constants (gpsimd iota/affine_select aren't available)
    ident_d = nc.dram_tensor("ident", [128, 128], F32, kind="ExternalInput").ap()
    slmat_d = nc.dram_tensor("slmat", [128, 128], F32, kind="ExternalInput").ap()
    tidx_d = nc.dram_tensor("tidx", [128, 64], F32, kind="ExternalInput").ap()
    eidx_d = nc.dram_tensor("eidx", [128, E], F32, kind="ExternalInput").ap()
    y_d = nc.dram_tensor("y", [SH, D], BF16, kind="ExternalOutput").ap()
    # token-space partial output; zero-filled on device early in the run
    # (collectives may not read IO tensors, so this must stay internal)
    part_d = nc.dram_tensor("part", [T + 1, D], BF16).ap()

    zsrc_d = nc.dram_tensor("zsrc", [SH, D], BF16, kind="ExternalInput").ap()

    # ---- internal DRAM ----
    pay_in = nc.dram_tensor("pay_in", [4 * SH], F32).ap()
    pay_all = nc.dram_tensor("pay_all", [NC * 4 * SH], F32, addr_space="Shared").ap()
    pay_tab = nc.dram_tensor("pay_tab", [T + 1, 4], F32).ap()
    rs_out = nc.dram_tensor("rs_out", [SH, D], BF16).ap()

    with tile.TileContext(nc) as tc:
        with (
            tc.tile_pool(name="persist", bufs=1) as pp,
            tc.tile_pool(name="psum_s", bufs=2, space="PSUM") as pss,
        ):
            # route pool is opened here (before the persist consts, so xT's
            # DMA is emitted first) and closed explicitly before the FFN to
            # free its SBUF
            _route_cm = tc.tile_pool(name="route", bufs=1)
            pr = _route_cm.__enter__()

            # xT is the head of the critical path: emit it before everything
            # else so it gets the first DMA slot
            xT_sb = pr.tile([128, D // 128, SH], F32)
            nc.sync.dma_start(xT_sb[:], xT_sh.rearrange("(o q) t -> q o t", q=128))
            wg_sb = pr.tile([128, D // 128, E], F32)
            nc.sync.dma_start(wg_sb[:], wg_d.rearrange("(o q) e -> q o e", q=128))

            # zero-fill the token-space partial buffer. The 1MB chunks chain
            # off each other (RAW on the previous chunk), so at most one is in
            # flight and later critical DMAs (payload write, AG rereads,
            # dispatch gathers) wait at most ~3us for the DMA engines. Must
            # complete before the first eo scatter (~150us).
            ZC = 512
            nc.scalar.dma_start(part_d[0:ZC, :], zsrc_d[0:ZC, :])
            for zc in range(1, T // ZC):
                nc.scalar.dma_start(
                    part_d[ZC * zc : ZC * (zc + 1), :],
                    part_d[ZC * (zc - 1) : ZC * zc, :],
                )

            ident = pp.tile([128, 128], F32)
            nc.sync.dma_start(ident[:], ident_d[:])
            ident_bf = pp.tile([128, 128], BF16)
            nc.vector.tensor_copy(ident_bf[:], ident[:])
            cid = pp.tile([128, 1], F32)
            nc.sync.dma_start(cid[:], cid_d[:])
            slotid = pp.tile([128, C // 128], F32)
            nc.sync.dma_start(slotid[:], slotid_d[:])
            zeros64 = pp.tile([128, 64], F32)
            nc.vector.memset(zeros64[:], 0.0)
            ones128 = pp.tile([128, 128], F32)
            nc.vector.memset(ones128[:], 1.0)

            # resident expert weights (bf16)
            wgt_sb = pp.tile([128, D // 128, F], BF16)
            wdn_sb = pp.tile([128, F // 128, D], BF16)



            # persistent routing products
            tokc = pp.tile([128, C // 128], I32)    # dispatch: slot->token, col-major
            slotg = pp.tile([128, C // 128], F32)   # gate per slot, col-major

            # =============== ROUTER (token shard, fp32) ===============
            if True:
                sl = pr.tile([128, 128], F32)
                nc.sync.dma_start(sl[:], slmat_d[:])
                tif = pr.tile([128, 64], F32)
                nc.sync.dma_start(tif[:], tidx_d[:])
                eidx = pr.tile([128, E], F32)
                nc.sync.dma_start(eidx[:], eidx_d[:])



                lg = pr.tile([128, 8, E], F32)  # logits, token pos j = 128*tt + p
                for tt in range(8):
                    ps = pss.tile([128, E], F32, space="PSUM", tag="ps_small")
                    for kd in range(8):
                        nc.tensor.matmul(
                            ps[:],
                            lhsT=xT_sb[:, kd, 128 * tt : 128 * tt + 128],
                            rhs=wg_sb[:, kd, :],
                            start=(kd == 0),
                            stop=(kd == 7),
                        )
                    nc.vector.tensor_copy(lg[:, tt, :], ps[:])

                m1x = pr.tile([128, 8], F32)
                nc.vector.tensor_reduce(m1x[:], lg[:], AX.X, OP.max)


                is1 = pr.tile([128, 8, E], F32)
                nc.vector.tensor_tensor(
                    out=is1[:], in0=lg[:], in1=m1x[:, :, None].to_broadcast([128, 8, E]),
                    op=OP.is_equal,
                )
                l2 = pr.tile([128, 8, E], F32)
                nc.vector.scalar_tensor_tensor(
                    out=l2[:], in0=is1[:], scalar=-1e30, in1=lg[:], op0=OP.mult, op1=OP.add,
                )
                m2x = pr.tile([128, 8], F32)
                nc.vector.tensor_reduce(m2x[:], l2[:], AX.X, OP.max)
                is2 = pr.tile([128, 8, E], F32)
                nc.vector.tensor_tensor(
                    out=is2[:], in0=l2[:], in1=m2x[:, :, None].to_broadcast([128, 8, E]),
                    op=OP.is_equal,
                )

                # argmax index = sum(mask * eidx) along E
                i1f = pr.tile([128, 8], F32)
                sc1a = pr.tile([128, 8, E], F32, tag="am_scr_a")
                nc.vector.tensor_tensor(
                    out=sc1a[:], in0=is1[:], in1=eidx[:, None, :].to_broadcast([128, 8, E]),
                    op=OP.mult,
                )
                nc.vector.tensor_reduce(i1f[:], sc1a[:], AX.X, OP.add)
                i2f = pr.tile([128, 8], F32)
                sc2a = pr.tile([128, 8, E], F32, tag="am_scr_b")
                nc.vector.tensor_tensor(
                    out=sc2a[:], in0=is2[:], in1=eidx[:, None, :].to_broadcast([128, 8, E]),
                    op=OP.mult,
                )
                nc.vector.tensor_reduce(i2f[:], sc2a[:], AX.X, OP.add)

                # top-2 softmax gates: g1 = 1/(1+exp(m2-m1)), g2 = 1-g1
                dm = pr.tile([128, 8], F32)
                nc.vector.tensor_tensor(out=dm[:], in0=m2x[:], in1=m1x[:], op=OP.subtract)
                e2 = pr.tile([128, 8], F32)
                nc.scalar.activation(e2[:], dm[:], AF.Exp)
                den = pr.tile([128, 8], F32)
                nc.vector.tensor_scalar_add(den[:], e2[:], 1.0)
                g1 = pr.tile([128, 8], F32)
                nc.vector.reciprocal(g1[:], den[:])
                g2 = pr.tile([128, 8], F32)
                nc.vector.tensor_tensor(out=g2[:], in0=e2[:], in1=g1[:], op=OP.mult)

                pk = pr.tile([128, 4, 8], F32)
                nc.vector.tensor_copy(pk[:, 0, :], i1f[:])
                nc.vector.tensor_copy(pk[:, 1, :], i2f[:])
                nc.vector.tensor_copy(pk[:, 2, :], g1[:])
                nc.vector.tensor_copy(pk[:, 3, :], g2[:])
                nc.sync.dma_start(pay_in.rearrange("(a p tt) -> p a tt", a=4, p=128), pk[:])

                # gate_proj weights: 2MB links, each gated on the previous via
                # a tiny DVE copy; the chain head hangs off the payload pack so
                # the stream starts right after the (critical) payload write
                # and never monopolizes the DMA engines
                wgt_v = wgt_d.rearrange("(o q) f -> q o f", q=128)
                for h in range(4):
                    src = pk[0:1, 3, 0:2] if h == 0 else wgt_sb[0:1, 2 * (h - 1), 0:2]
                    nc.vector.tensor_copy(wgt_sb[0:1, 2 * h, 0:2], src)
                    nc.sync.dma_start(
                        wgt_sb[:, 2 * h : 2 * (h + 1), :], wgt_v[:, 2 * h : 2 * (h + 1), :]
                    )

                nc.gpsimd.collective_compute(
                    "AllGather", OP.bypass,
                    replica_groups=[list(range(NC))],
                    ins=[pay_in[:].opt()], outs=[pay_all[:].opt()],
                )

                nc.scalar.dma_start(pay_tab[T : T + 1, :], zeros64[0:1, 0:4])

                # reread all 4 arrays into global routing layout [128, 64]
                # (t = 64p + i). NOTE: a merged/rearranged form is NOT safe
                # here — splitting the SBUF partition dim (e.g. "(r p16) i ->
                # r p16 i") silently drops partition semantics (partition_size
                # becomes r) and writes garbage on hardware.
                rt = pr.tile([128, 4, 64], F32)
                pay_view = pay_all.rearrange("(r a p16 i) -> r p16 a i", r=NC, a=4, p16=16)
                for r in range(NC):
                    nc.sync.dma_start(rt[16 * r : 16 * r + 16, :, :], pay_view[r])
                i1r, i2r = rt[:, 0, :], rt[:, 1, :]
                g1r, g2r = rt[:, 2, :], rt[:, 3, :]



                # =============== SLOT ASSIGNMENT (replicated) ===============
                # (the real compiler only allows generic vector ops on DVE,
                # so both choice chains share it)
                v1, v2 = nc.vector, nc.vector
                m1 = pr.tile([128, E, 64], F32)
                m2 = pr.tile([128, E, 64], F32)
                sc1 = pr.tile([128, E, 64], F32)
                sc2 = pr.tile([128, E, 64], F32)
                v1.tensor_tensor(
                    out=m1[:], in0=i1r[:, None, :].to_broadcast([128, E, 64]),
                    in1=eidx[:, :, None].to_broadcast([128, E, 64]), op=OP.is_equal,
                )
                v2.tensor_tensor(
                    out=m2[:], in0=i2r[:, None, :].to_broadcast([128, E, 64]),
                    in1=eidx[:, :, None].to_broadcast([128, E, 64]), op=OP.is_equal,
                )
                for e in range(E):
                    v1.tensor_tensor_scan(sc1[:, e, :], m1[:, e, :], zeros64[:], 0.0, op0=OP.add, op1=OP.add)
                    v2.tensor_tensor_scan(sc2[:, e, :], m2[:, e, :], zeros64[:], 0.0, op0=OP.add, op1=OP.add)
                tot1 = pr.tile([128, E], F32)
                tot2 = pr.tile([128, E], F32)
                v1.tensor_copy(tot1[:], sc1[:, :, 63])
                v2.tensor_copy(tot2[:], sc2[:, :, 63])

                of1_ps = pss.tile([128, E], F32, space="PSUM", tag="ps_small")
                nc.tensor.matmul(of1_ps[:], lhsT=sl[:], rhs=tot1[:], start=True, stop=True)
                of1 = pr.tile([128, E], F32)
                nc.vector.tensor_scalar_add(of1[:], of1_ps[:], -1.0)
                of2_ps = pss.tile([128, E], F32, space="PSUM", tag="ps_small")
                nc.tensor.matmul(of2_ps[:], lhsT=sl[:], rhs=tot2[:], start=True, stop=False)
                nc.tensor.matmul(of2_ps[:], lhsT=ones128[:], rhs=tot1[:], start=False, stop=True)
                of2 = pr.tile([128, E], F32)
                nc.vector.tensor_scalar_add(of2[:], of2_ps[:], -1.0)

                def loc_s(vv, sc, m, of, tag):
                    tmp = pr.tile([128, E, 64], F32, tag=f"loc_tmp{tag}")
                    for e in range(E):
                        vv.scalar_tensor_tensor(
                            out=tmp[:, e, :], in0=sc[:, e, :], scalar=of[:, e : e + 1],
                            in1=m[:, e, :], op0=OP.add, op1=OP.mult,
                        )
                    cur, w = tmp, E
                    while w > 1:
                        nxt = pr.tile([128, w // 2, 64], F32, tag=f"loc_s{tag}{w}")
                        vv.tensor_tensor(out=nxt[:], in0=cur[:, : w // 2, :], in1=cur[:, w // 2 :, :], op=OP.add)
                        cur, w = nxt, w // 2
                    return cur  # [128, 1, 64]

                l1s = loc_s(v1, sc1, m1, of1, "a")[:, 0, :]
                l2s = loc_s(v2, sc2, m2, of2, "b")[:, 0, :]

                def keep_f(vv, ls, ir, tag):
                    kp = pr.tile([128, 64], F32, tag=f"kp{tag}")
                    vv.tensor_scalar(out=kp[:], in0=ls, scalar1=float(C), scalar2=None, op0=OP.is_lt)
                    lc = pr.tile([128, 64], F32, tag=f"lc{tag}")
                    vv.tensor_scalar(out=lc[:], in0=ls, scalar1=float(C - 1), scalar2=None, op0=OP.min)
                    f = pr.tile([128, 64], F32, tag=f"f{tag}")
                    vv.scalar_tensor_tensor(out=f[:], in0=ir, scalar=float(C), in1=lc[:], op0=OP.mult, op1=OP.add)
                    return f, kp

                f1, kp1 = keep_f(v1, l1s, i1r, "a")
                f2, kp2 = keep_f(v2, l2s, i2r, "b")

                # payload table rows t = 64p + i: (f1, f2, g1, g2)
                pt_sb = pr.tile([128, 64, 4], F32)
                nc.vector.tensor_copy(pt_sb[:, :, 0], f1[:])
                nc.vector.tensor_copy(pt_sb[:, :, 1], f2[:])
                nc.vector.tensor_copy(pt_sb[:, :, 2], g1r)
                nc.vector.tensor_copy(pt_sb[:, :, 3], g2r)
                nc.sync.dma_start(
                    pay_tab[0:T, :].rearrange("(p i) c -> p i c", p=128), pt_sb[:]
                )

                # ====== SLOT -> TOKEN MAP (local_scatter + merge + diagonal) ======
                tp1 = pr.tile([128, 64], F32)
                nc.vector.tensor_scalar_add(tp1[:], tif[:], 1.0)   # token id + 1

                def slot_halves(vv, ls, ir, kp, tag):
                    # sel = (expert == cid) && kept; slot+1 where selected else 0
                    isc = pr.tile([128, 64], F32, tag=f"isc{tag}")
                    vv.tensor_tensor(out=isc[:], in0=ir, in1=cid[:, 0:1].to_broadcast([128, 64]), op=OP.is_equal)
                    sel = pr.tile([128, 64], F32, tag=f"sel{tag}")
                    vv.tensor_tensor(out=sel[:], in0=isc[:], in1=kp[:], op=OP.mult)
                    sp1 = pr.tile([128, 64], F32, tag=f"sp1{tag}")  # sel ? slot+1 : 0
                    vv.tensor_scalar_add(sp1[:], ls, 1.0)
                    vv.tensor_tensor(out=sp1[:], in0=sp1[:], in1=sel[:], op=OP.mult)
                    # lo half: slot in [0, 1024): idx = slot, else -1
                    mlo = pr.tile([128, 64], F32, tag=f"mlo{tag}")
                    vv.tensor_scalar(out=mlo[:], in0=sp1[:], scalar1=1024.0, scalar2=None, op0=OP.is_le)
                    vv.tensor_tensor(out=mlo[:], in0=mlo[:], in1=sel[:], op=OP.mult)
                    ilo = pr.tile([128, 64], F32, tag=f"ilo{tag}")
                    vv.tensor_tensor(out=ilo[:], in0=mlo[:], in1=sp1[:], op=OP.mult)
                    vv.tensor_scalar_add(ilo[:], ilo[:], -1.0)
                    # hi half: slot in [1024, 2048): idx = slot - 1024, else -1
                    mhi = pr.tile([128, 64], F32, tag=f"mhi{tag}")
                    vv.tensor_scalar(out=mhi[:], in0=sp1[:], scalar1=1024.0, scalar2=None, op0=OP.is_gt)
                    ihi = pr.tile([128, 64], F32, tag=f"ihi{tag}")
                    vv.tensor_scalar_add(ihi[:], sp1[:], -1024.0)
                    vv.tensor_tensor(out=ihi[:], in0=ihi[:], in1=mhi[:], op=OP.mult)
                    vv.tensor_scalar_add(ihi[:], ihi[:], -1.0)
                    return ilo, ihi

                i1lo, i1hi = slot_halves(v1, l1s, i1r, kp1, "a")
                i2lo, i2hi = slot_halves(v2, l2s, i2r, kp2, "b")

                data128 = pr.tile([128, 128], I16)
                v1.tensor_copy(data128[:, :64], tp1[:])
                v2.tensor_copy(data128[:, 64:], tp1[:])
                idxlo = pr.tile([128, 128], I16)
                v1.tensor_copy(idxlo[:, :64], i1lo[:])
                v2.tensor_copy(idxlo[:, 64:], i2lo[:])
                idxhi = pr.tile([128, 128], I16)
                v1.tensor_copy(idxhi[:, :64], i1hi[:])
                v2.tensor_copy(idxhi[:, 64:], i2hi[:])

                dst_lo = pr.tile([128, 1024], I16)
                nc.gpsimd.local_scatter(dst_lo[:], data128[:], idxlo[:], channels=128, num_elems=1024, num_idxs=128)
                dst_hi = pr.tile([128, 1024], I16)
                nc.gpsimd.local_scatter(dst_hi[:], data128[:], idxhi[:], channels=128, num_elems=1024, num_idxs=128)

                # merge across partitions: each slot column has at most one
                # nonzero writer, so a gpsimd partition all-reduce (max)
                # replicates the slot->token map onto every partition
                merged = pr.tile([128, 2, 1024], F32)  # map+1 on all partitions
                nc.gpsimd.partition_all_reduce(
                    merged[:, 0, :], dst_lo[:], channels=128, reduce_op=bass_isa.ReduceOp.max
                )
                nc.gpsimd.partition_all_reduce(
                    merged[:, 1, :], dst_hi[:], channels=128, reduce_op=bass_isa.ReduceOp.max
                )

                # diagonal extraction: tokraw[p, k] = merged-flat[128k + p]
                tokraw = pr.tile([128, C // 128], F32)
                scratch = pr.tile([128, 128], F32, tag="diag_scr")
                mview = merged[:].rearrange("p a b -> p (a b)")
                scratch2 = pr.tile([128, 128], F32, tag="diag_scr2")
                iszero = pr.tile([128, C // 128], F32)

                def diag_cols(k0, k1):
                    # extract columns [k0,k1), sanitize (0 -> T+1; v -> v-1),
                    # and publish them to tokc so dependent gathers can start
                    for k in range(k0, k1):
                        vv, scr = (v1, scratch) if k % 2 == 0 else (v2, scratch2)
                        vv.scalar_tensor_tensor(
                            out=scr[:], in0=mview[:, 128 * k : 128 * (k + 1)], scalar=0.0,
                            in1=ident[:], op0=OP.add, op1=OP.mult,
                            accum_out=tokraw[:, k : k + 1],
                        )
                    nc.vector.tensor_scalar(out=iszero[:, k0:k1], in0=tokraw[:, k0:k1], scalar1=0.0, scalar2=None, op0=OP.is_equal)
                    nc.vector.scalar_tensor_tensor(
                        out=tokraw[:, k0:k1], in0=iszero[:, k0:k1], scalar=float(T + 1),
                        in1=tokraw[:, k0:k1], op0=OP.mult, op1=OP.add,
                    )
                    nc.vector.tensor_scalar_add(tokraw[:, k0:k1], tokraw[:, k0:k1], -1.0)
                    nc.vector.tensor_copy(tokc[:, k0:k1], tokraw[:, k0:k1])

                # cb0's dispatch gathers need only the first 4 columns: emit
                # them first so the FFN pipeline starts while the rest extract
                diag_cols(0, CBLK // 128)
                diag_cols(CBLK // 128, C // 128)

                # down_proj weights: 2MB links chained like wgt, head gated on
                # the slot map; needed only by cb0's mm2 (~140us)
                wdn_v = wdn_d.rearrange("(o q) d -> q o d", q=128)
                for h in range(4):
                    src = tokraw[0:1, 0:2] if h == 0 else wdn_sb[0:1, 8 * (h - 1), 0:2]
                    nc.vector.tensor_copy(wdn_sb[0:1, 8 * h, 0:2], src)
                    nc.sync.dma_start(
                        wdn_sb[:, 8 * h : 8 * (h + 1), :], wdn_v[:, 8 * h : 8 * (h + 1), :]
                    )

            _route_cm.__exit__(None, None, None)

            # =============== EXPERT FFN (bf16) ===============
            with (
                tc.tile_pool(name="ffn", bufs=1) as pf,
                tc.tile_pool(name="ffn_db", bufs=2) as pfd,
                tc.tile_pool(name="ffn_dr", bufs=4) as pdr,
                tc.tile_pool(name="psum_mm", bufs=2, space="PSUM") as psm,
            ):
                def emit_dispatch(cb):
                    # gather 4 x 128 slot rows and transpose into dispT.
                    # cb0 is latency-critical and the DMA device is still busy
                    # with the weight/zero chains, so it uses PE transposes;
                    # the prefetched blocks (huge slack) use ONE xbar DMA
                    # transpose per row block instead, taking ~5us of
                    # transposes+copies off the PE/DVE steady state.
                    KT = CBLK // 128
                    dispT = pfd.tile([128, D // 128, CBLK], BF16, tag="dispT")
                    for kt in range(KT):
                        k = KT * cb + kt
                        drow = pdr.tile([128, D], BF16, tag="drow")
                        nc.gpsimd.indirect_dma_start(
                            out=drow[:], out_offset=None, in_=xb[:],
                            in_offset=bass.IndirectOffsetOnAxis(ap=tokc[:, k : k + 1], axis=0),
                        )
                        if cb == 0:
                            for dt in range(D // 128):
                                tr_ps = psm.tile([128, 128], BF16, space="PSUM", tag="ps_tr")
                                nc.tensor.transpose(tr_ps[:], drow[:, 128 * dt : 128 * (dt + 1)], ident_bf[:])
                                nc.vector.tensor_copy(dispT[:, dt, 128 * kt : 128 * (kt + 1)], tr_ps[:])
                        else:
                            nc.sync.dma_start_transpose(
                                dispT[:, :, 128 * kt : 128 * (kt + 1)], drow[:]
                            )
                    return dispT

                next_dispT = emit_dispatch(0)
                for cb in range(NCB):
                    dispT = next_dispT

                    if cb == 0:
                        # slot gates: gather payload rows by slot owner, then
                        # gate = (f1==slot)*g1 + (f2==slot)*g2. Emitted after
                        # cb0's dispatch gathers so they don't delay the FFN
                        # start on the (in-order) gpsimd queue; results are
                        # only needed by cb0's mm2 scale, ~100us later.
                        pg = pf.tile([128, C // 128, 4], F32, tag="pg")
                        for k in range(C // 128):
                            nc.gpsimd.indirect_dma_start(
                                out=pg[:, k, :], out_offset=None, in_=pay_tab[:],
                                in_offset=bass.IndirectOffsetOnAxis(ap=tokc[:, k : k + 1], axis=0),
                            )
                        is1g = pf.tile([128, C // 128], F32, tag="is1g")
                        nc.vector.tensor_tensor(out=is1g[:], in0=pg[:, :, 0], in1=slotid[:], op=OP.is_equal)
                        is2g = pf.tile([128, C // 128], F32, tag="is2g")
                        nc.vector.tensor_tensor(out=is2g[:], in0=pg[:, :, 1], in1=slotid[:], op=OP.is_equal)
                        ga = pf.tile([128, C // 128], F32, tag="ga")
                        nc.vector.tensor_tensor(out=ga[:], in0=is1g[:], in1=pg[:, :, 2], op=OP.mult)
                        gb = pf.tile([128, C // 128], F32, tag="gb")
                        nc.vector.tensor_tensor(out=gb[:], in0=is2g[:], in1=pg[:, :, 3], op=OP.mult)
                        nc.vector.tensor_tensor(out=slotg[:], in0=ga[:], in1=gb[:], op=OP.add)

                    hT = pf.tile([128, F // 128, CBLK], BF16, tag="hT")
                    for ft in range(F // 128):
                        ps1 = psm.tile([128, CBLK], F32, space="PSUM", tag="ps1")
                        for kd in range(D // 128):
                            nc.tensor.matmul(
                                ps1[:],
                                lhsT=wgt_sb[:, kd, 128 * ft : 128 * ft + 128],
                                rhs=dispT[:, kd, :],
                                start=(kd == 0), stop=(kd == D // 128 - 1),
                            )
                        nc.scalar.activation(hT[:, ft, :], ps1[:], AF.Gelu)

                    # prefetch the next block's dispatch AFTER this block's mm1
                    # emission: Tile's in-order PE queue then runs those
                    # transposes only when their gathers are long done, instead
                    # of idling PE mid-mm1 waiting for them
                    if cb + 1 < NCB:
                        next_dispT = emit_dispatch(cb + 1)

                    # mm2 with swapped operands: eo[c, d] = hT.T @ w_down -> row-major out
                    # PSUM->SBUF copy applies the slot gate (per-partition scalar)
                    eo_sb = pf.tile([128, CBLK // 128, D], BF16, tag="eo_sb")
                    for ct in range(CBLK // 128):
                        k = (CBLK // 128) * cb + ct
                        for dc in range(D // 512):
                            ps2 = psm.tile([128, 512], F32, space="PSUM", tag="ps2")
                            for ft in range(F // 128):
                                nc.tensor.matmul(
                                    ps2[:],
                                    lhsT=hT[:, ft, 128 * ct : 128 * ct + 128],
                                    rhs=wdn_sb[:, ft, 512 * dc : 512 * dc + 512],
                                    start=(ft == 0), stop=(ft == F // 128 - 1),
                                )
                            nc.vector.tensor_scalar_mul(
                                eo_sb[:, ct, 512 * dc : 512 * dc + 512], ps2[:],
                                slotg[:, k : k + 1],
                            )
                        # scatter scaled rows into token space (trash row T for
                        # empty slots; their eo is exactly 0 anyway). The static
                        # out AP is a 128-row window (offset must be 0): the
                        # actual rows come from the dynamic offsets, but the
                        # cost model (and descriptor count) key on the static
                        # AP, which must not span the whole 16.8MB tensor.
                        nc.gpsimd.indirect_dma_start(
                            out=part_d[0:T, :].rearrange("(a b) d -> a (b d)", b=64)[:, 0:D],
                            out_offset=bass.IndirectOffsetOnAxis(ap=tokc[:, k : k + 1], axis=0),
                            in_=eo_sb[:, ct, :], in_offset=None,
                        )

            # =============== COMBINE: ReduceScatter over token space ===============
            # (collectives may not read or write IO tensors: internal in/out,
            # then a small DMA moves the reduced shard to y)
            nc.gpsimd.collective_compute(
                "ReduceScatter", OP.add,
                replica_groups=[list(range(NC))],
                ins=[part_d[0:T, :].opt()], outs=[rs_out[:].opt()],
            )
            nc.sync.dma_start(y_d[:], rs_out[:])

    nc.compile()
    return nc


_PROGRAM = None


def _get_program():
    global _PROGRAM
    if _PROGRAM is None:
        _PROGRAM = _build_program()
    return _PROGRAM


def host_constants():
    p = np.arange(128)
    return {
        "ident": np.eye(128, dtype=np.float32),
        "slmat": (np.arange(128)[None, :] > p[:, None]).astype(np.float32),
        "tidx": (64 * p[:, None] + np.arange(64)[None, :]).astype(np.float32),
        "eidx": np.tile(np.arange(E, dtype=np.float32), (128, 1)),
    }


def _make_in_maps(x, wg, w_gate, w_down):
    x = np.asarray(x, np.float32)
    wg_np = np.asarray(wg, np.float32)
    w_gate_np = np.asarray(w_gate, np.float32)
    w_down_np = np.asarray(w_down, np.float32)

    tokens = x.reshape(T, D)
    xb = np.zeros((T + 1, D), ml_dtypes.bfloat16)
    xb[:T] = tokens.astype(ml_dtypes.bfloat16)

    # shard m holds tokens [SH*m, SH*(m+1)); its xT columns are permuted so that
    # matmul tile position j = 128*tt + p corresponds to local token 8*p + tt,
    # making the routing payload DMA contiguous.
    j = np.arange(SH)
    perm = 8 * (j % 128) + j // 128  # local token index at column position j
    consts = host_constants()
    p = np.arange(128)
    kk = np.arange(C // 128)

    in_maps = []
    for m in range(NC):
        shard = tokens[SH * m : SH * (m + 1)]
        xT_sh = np.ascontiguousarray(shard[perm].T)
        in_maps.append({
            "xT_sh": xT_sh,
            "xb": xb,
            "wg": wg_np,
            "wgt": np.ascontiguousarray(w_gate_np[m].astype(ml_dtypes.bfloat16)),
            "wdn": np.ascontiguousarray(w_down_np[m].astype(ml_dtypes.bfloat16)),
            "cid": np.full((128, 1), float(m), np.float32),
            "slotid": (m * C + 128 * kk[None, :] + p[:, None]).astype(np.float32),
            "zsrc": np.zeros((SH, D), ml_dtypes.bfloat16),
            **consts,
        })
    return in_maps


def kernel(x, wg, w_gate, w_down, _trace=False):
    global LAST_RESULT
    x = np.asarray(x, np.float32)
    in_maps = _make_in_maps(x, wg, w_gate, w_down)

    nc = _get_program()
    res = run_bass_kernel_spmd(nc, in_maps, core_ids=list(range(NC)), trace=_trace)
    LAST_RESULT = res
    out = np.concatenate([res.results[m]["y"] for m in range(NC)], axis=0)
    return out.reshape(B, S, D).astype(x.dtype)


def bench(x, wg, w_gate, w_down, iters=6):
    """Measure per-execution wall time with device-resident inputs.

    Returns (output, per_call_seconds_list) where each call gets freshly
    zeroed (donated) output buffers, matching run_bass_via_pjrt semantics.
    """
    import time
    import jax
    from jax.sharding import Mesh, PartitionSpec, NamedSharding
    from jax.experimental.shard_map import shard_map
    import concourse.mybir as _mybir
    from concourse.bass2jax import _bass_exec_p, install_neuronx_cc_hook, partition_id_tensor

    install_neuronx_cc_hook()
    nc = _get_program()

    x = np.asarray(x, np.float32)
    in_maps = _make_in_maps(x, wg, w_gate, w_down)

    in_names, out_names, out_avals, zero_outs = [], [], [], []
    for alloc in nc.m.functions[0].allocations:
        if not isinstance(alloc, _mybir.MemoryLocationSet):
            continue
        name = alloc.memorylocations[0].name
        if alloc.kind == "ExternalInput":
            if nc.partition_id_tensor is None or name != nc.partition_id_tensor.name:
                in_names.append(name)
        elif alloc.kind == "ExternalOutput":
            shape = tuple(alloc.tensor_shape)
            dtype = _mybir.dt.np(alloc.dtype)
            out_names.append(name)
            out_avals.append(jax.core.ShapedArray(shape, dtype))
            zero_outs.append(np.zeros(shape, dtype))
    n_params = len(in_names)
    all_in_names = in_names + out_names
    if nc.partition_id_tensor is not None:
        all_in_names = all_in_names + [nc.partition_id_tensor.name]

    def _body(*args):
        operands = list(args)
        if nc.partition_id_tensor is not None:
            operands.append(partition_id_tensor())
        outs = _bass_exec_p.bind(
            *operands,
            out_avals=tuple(out_avals),
            in_names=tuple(all_in_names),
            out_names=tuple(out_names),
            lowering_input_output_aliases=(),
            sim_require_finite=True,
            sim_require_nnan=True,
            nc=nc,
        )
        return tuple(outs)

    devices = jax.devices()[:NC]
    mesh = Mesh(np.asarray(devices), ("core",))
    nsh = NamedSharding(mesh, PartitionSpec("core"))
    n_outs = len(out_avals)
    donate = tuple(range(n_params, n_params + n_outs))
    sharded = jax.jit(
        shard_map(_body, mesh=mesh, in_specs=(PartitionSpec("core"),) * (n_params + n_outs),
                  out_specs=(PartitionSpec("core"),) * n_outs, check_rep=False),
        donate_argnums=donate, keep_unused=True,
    )

    concat_in = [
        jax.device_put(np.concatenate([np.asarray(in_maps[c][nm]) for c in range(NC)], axis=0), nsh)
        for nm in in_names
    ]
    zero_sets = [
        [jax.device_put(np.zeros((NC * z.shape[0], *z.shape[1:]), z.dtype), nsh) for z in zero_outs]
        for _ in range(iters + 1)
    ]

    out = sharded(*concat_in, *zero_sets[0])  # warmup + compile
    jax.block_until_ready(out)
    times = []
    for it in range(iters):
        t0 = time.perf_counter()
        out = sharded(*concat_in, *zero_sets[it + 1])
        jax.block_until_ready(out)
        times.append(time.perf_counter() - t0)

    outs = {
        nm: np.asarray(out[i]).reshape(NC, *out_avals[i].shape) for i, nm in enumerate(out_names)
    }
    y = np.concatenate([outs["y"][m] for m in range(NC)], axis=0).reshape(B, S, D).astype(x.dtype)
    return y, times


# revision 89
# speedup vs baseline: 1.0414x; 1.0176x over previous
"""MoE layer (GShard top-2 routing + per-expert FFN) on 8 Trainium2 NeuronCores.

Strategy (expert parallelism, ReduceScatter combine):
  - Router matmul (fp32, exact) is token-sharded: each core computes logits for
    its 1024-token shard, then an AllGather shares per-token routing scalars
    (idx1, idx2, g1, g2) with all cores.
  - Every core replicates the (cheap) global slot-assignment math: per-expert
    inclusive scans along the free dim + a triangular-matmul partition prefix
    give each token its capacity slot exactly as the reference's cumsum does.
  - Each core owns ONE expert. The slot->token map is built with local_scatter
    (per-partition scatter of token ids by slot), merged across partitions
    with a gpsimd partition all-reduce (each slot column has one writer), and
    read out column-major via a diagonal extraction (first 4 columns early so
    cb0's dispatch gathers start while the rest extract).
  - Dispatch: 16 indirect row gathers from x (bf16) + PE transposes give the
    [d, slot] layout; FFN in bf16 with fp32 accumulation:
    hT = gelu(w_gate^T @ dispT), eo = hT^T @ w_down (row-major out).
  - Combine via ReduceScatter: a [T+1,4] payload table (f1,f2,g1,g2 per token)
    is written to DRAM and gathered by the slot->token map, giving each slot
    its owner's gate. mm2's PSUM->SBUF copy scales eo rows by that gate, and
    the scaled rows are indirect-scattered into a [T+1,D] bf16 token-space
    partial buffer ("part", zero-filled on device early in the run; the
    collective verifier forbids IO tensors, so it must stay internal). A
    single bf16 ReduceScatter(add) over part[0:T] then yields each core's
    final output shard directly (tokens are shard-ordered), bounced to y
    (bf16) and cast to fp32 on the host.

  Scheduling notes (the TimelineSim cost model serializes all DMA on one
  device, FIFO by acquire time, and Tile schedules by dependency, not
  program order):
  - Big loads (weights, zero-fill) run as single-in-flight chains: each link
    is gated on the previous via a tiny DVE copy (weights) or a RAW
    self-copy (zero chunks), so routing-critical DMAs (payload write, AG
    rereads, dispatch gathers) never wait more than one ~3-6us link.
  - Indirect scatters claim a strided static window (rows 0,64,...,8128) of
    "part": cost is charged on the static AP (256KB, not 16.8MB), while the
    window still overlaps every zero chunk so Tile orders all scatters after
    the zero fill. Do NOT "slice" SBUF partition dims via rearrange in DMA
    APs (e.g. "(r p16) i -> r p16 i") — partition_size silently becomes r
    and the transfer writes garbage on hardware.
"""

import sys

if "/opt/trn_rl_repo" not in sys.path:
    sys.path.insert(0, "/opt/trn_rl_repo")

import numpy as np
import ml_dtypes

import concourse.bacc as bacc
import concourse.mybir as mybir
import concourse.tile as tile
from concourse import bass
from concourse import bass_isa
from concourse.bass_utils import run_bass_kernel_spmd

BF16 = mybir.dt.bfloat16
F32 = mybir.dt.float32
I16 = mybir.dt.int16
I32 = mybir.dt.int32
AF = mybir.ActivationFunctionType
OP = mybir.AluOpType
AX = mybir.AxisListType

B, S, D, E, F = 4, 2048, 1024, 8, 4096
T = B * S            # 8192 tokens
C = 2 * T // E       # 2048 capacity
NC = 8               # cores
SH = T // NC         # 1024 tokens per shard
CBLK = 512           # FFN slot-block
NCB = C // CBLK      # 4 blocks

LAST_RESULT = None   # BassKernelResults of the most recent run (for profiling)


def _build_program():
    nc = bacc.Bacc("TRN2", target_bir_lowering=False, debug=False, num_devices=NC)

    # ---- per-core external inputs ----
    xT_sh = nc.dram_tensor("xT_sh", [D, SH], F32, kind="ExternalInput").ap()
    xb = nc.dram_tensor("xb", [T + 1, D], BF16, kind="ExternalInput").ap()
    wg_d = nc.dram_tensor("wg", [D, E], F32, kind="ExternalInput").ap()
    wgt_d = nc.dram_tensor("wgt", [D, F], BF16, kind="ExternalInput").ap()
    wdn_d = nc.dram_tensor("wdn", [F, D], BF16, kind="ExternalInput").ap()
    cid_d = nc.dram_tensor("cid", [128, 1], F32, kind="ExternalInput").ap()
    slotid_d = nc.dram_tensor("slotid", [128, C // 128], F32, kind="ExternalInput").ap()
    # host-generated constants (gpsimd iota/affine_select aren't available)
    ident_d = nc.dram_tensor("ident", [128, 128], F32, kind="ExternalInput").ap()
    slmat_d = nc.dram_tensor("slmat", [128, 128], F32, kind="ExternalInput").ap()
    tidx_d = nc.dram_tensor("tidx", [128, 64], F32, kind="ExternalInput").ap()
    eidx_d = nc.dram_tensor("eidx", [128, E], F32, kind="ExternalInput").ap()
    y_d = nc.dram_tensor("y", [SH, D], BF16, kind="ExternalOutput").ap()
    # token-space partial output; zero-filled on device early in the run
    # (collectives may not read IO tensors, so this must stay internal)
    part_d = nc.dram_tensor("part", [T + 1, D], BF16).ap()

    zsrc_d = nc.dram_tensor("zsrc", [SH, D], BF16, kind="ExternalInput").ap()

    # ---- internal DRAM ----
    pay_in = nc.dram_tensor("pay_in", [4 * SH], F32).ap()
    pay_all = nc.dram_tensor("pay_all", [NC * 4 * SH], F32, addr_space="Shared").ap()
    pay_tab = nc.dram_tensor("pay_tab", [T + 1, 4], F32).ap()
    rs_out = nc.dram_tensor("rs_out", [SH, D], BF16).ap()

    with tile.TileContext(nc) as tc:
        with (
            tc.tile_pool(name="persist", bufs=1) as pp,
            tc.tile_pool(name="psum_s", bufs=2, space="PSUM") as pss,
        ):
            # route pool is opened here (before the persist consts, so xT's
            # DMA is emitted first) and closed explicitly before the FFN to
            # free its SBUF
            _route_cm = tc.tile_pool(name="route", bufs=1)
            pr = _route_cm.__enter__()

            # xT is the head of the critical path: emit it before everything
            # else so it gets the first DMA slot
            xT_sb = pr.tile([128, D // 128, SH], F32)
            nc.sync.dma_start(xT_sb[:], xT_sh.rearrange("(o q) t -> q o t", q=128))
            wg_sb = pr.tile([128, D // 128, E], F32)
            nc.sync.dma_start(wg_sb[:], wg_d.rearrange("(o q) e -> q o e", q=128))

            # zero-fill the token-space partial buffer. The 1MB chunks chain
            # off each other (RAW on the previous chunk), so at most one is in
            # flight and later critical DMAs (payload write, AG rereads,
            # dispatch gathers) wait at most ~3us for the DMA engines. Must
            # complete before the first eo scatter (~150us).
            ZC = 512
            nc.scalar.dma_start(part_d[0:ZC, :], zsrc_d[0:ZC, :])
            for zc in range(1, T // ZC):
                nc.scalar.dma_start(
                    part_d[ZC * zc : ZC * (zc + 1), :],
                    part_d[ZC * (zc - 1) : ZC * zc, :],
                )

            ident = pp.tile([128, 128], F32)
            nc.sync.dma_start(ident[:], ident_d[:])
            ident_bf = pp.tile([128, 128], BF16)
            nc.vector.tensor_copy(ident_bf[:], ident[:])
            cid = pp.tile([128, 1], F32)
            nc.sync.dma_start(cid[:], cid_d[:])
            slotid = pp.tile([128, C // 128], F32)
            nc.sync.dma_start(slotid[:], slotid_d[:])
            zeros64 = pp.tile([128, 64], F32)
            nc.vector.memset(zeros64[:], 0.0)
            ones128 = pp.tile([128, 128], F32)
            nc.vector.memset(ones128[:], 1.0)

            # resident expert weights (bf16)
            wgt_sb = pp.tile([128, D // 128, F], BF16)
            wdn_sb = pp.tile([128, F // 128, D], BF16)



            # persistent routing products
            tokc = pp.tile([128, C // 128], I32)    # dispatch: slot->token, col-major
            slotg = pp.tile([128, C // 128], F32)   # gate per slot, col-major

            # =============== ROUTER (token shard, fp32) ===============
            if True:
                sl = pr.tile([128, 128], F32)
                nc.sync.dma_start(sl[:], slmat_d[:])
                tif = pr.tile([128, 64], F32)
                nc.sync.dma_start(tif[:], tidx_d[:])
                eidx = pr.tile([128, E], F32)
                nc.sync.dma_start(eidx[:], eidx_d[:])



                lg = pr.tile([128, 8, E], F32)  # logits, token pos j = 128*tt + p
                for tt in range(8):
                    ps = pss.tile([128, E], F32, space="PSUM", tag="ps_small")
                    for kd in range(8):
                        nc.tensor.matmul(
                            ps[:],
                            lhsT=xT_sb[:, kd, 128 * tt : 128 * tt + 128],
                            rhs=wg_sb[:, kd, :],
                            start=(kd == 0),
                            stop=(kd == 7),
                        )
                    nc.vector.tensor_copy(lg[:, tt, :], ps[:])

                m1x = pr.tile([128, 8], F32)
                nc.vector.tensor_reduce(m1x[:], lg[:], AX.X, OP.max)


                is1 = pr.tile([128, 8, E], F32)
                nc.vector.tensor_tensor(
                    out=is1[:], in0=lg[:], in1=m1x[:, :, None].to_broadcast([128, 8, E]),
                    op=OP.is_equal,
                )
                l2 = pr.tile([128, 8, E], F32)
                nc.vector.scalar_tensor_tensor(
                    out=l2[:], in0=is1[:], scalar=-1e30, in1=lg[:], op0=OP.mult, op1=OP.add,
                )
                m2x = pr.tile([128, 8], F32)
                nc.vector.tensor_reduce(m2x[:], l2[:], AX.X, OP.max)
                is2 = pr.tile([128, 8, E], F32)
                nc.vector.tensor_tensor(
                    out=is2[:], in0=l2[:], in1=m2x[:, :, None].to_broadcast([128, 8, E]),
                    op=OP.is_equal,
                )

                # argmax index = sum(mask * eidx) along E
                i1f = pr.tile([128, 8], F32)
                sc1a = pr.tile([128, 8, E], F32, tag="am_scr_a")
                nc.vector.tensor_tensor(
                    out=sc1a[:], in0=is1[:], in1=eidx[:, None, :].to_broadcast([128, 8, E]),
                    op=OP.mult,
                )
                nc.vector.tensor_reduce(i1f[:], sc1a[:], AX.X, OP.add)
                i2f = pr.tile([128, 8], F32)
                sc2a = pr.tile([128, 8, E], F32, tag="am_scr_b")
                nc.vector.tensor_tensor(
                    out=sc2a[:], in0=is2[:], in1=eidx[:, None, :].to_broadcast([128, 8, E]),
                    op=OP.mult,
                )
                nc.vector.tensor_reduce(i2f[:], sc2a[:], AX.X, OP.add)

                # top-2 softmax gates: g1 = 1/(1+exp(m2-m1)), g2 = 1-g1
                dm = pr.tile([128, 8], F32)
                nc.vector.tensor_tensor(out=dm[:], in0=m2x[:], in1=m1x[:], op=OP.subtract)
                e2 = pr.tile([128, 8], F32)
                nc.scalar.activation(e2[:], dm[:], AF.Exp)
                den = pr.tile([128, 8], F32)
                nc.vector.tensor_scalar_add(den[:], e2[:], 1.0)
                g1 = pr.tile([128, 8], F32)
                nc.vector.reciprocal(g1[:], den[:])
                g2 = pr.tile([128, 8], F32)
                nc.vector.tensor_tensor(out=g2[:], in0=e2[:], in1=g1[:], op=OP.mult)

                pk = pr.tile([128, 4, 8], F32)
                nc.vector.tensor_copy(pk[:, 0, :], i1f[:])
                nc.vector.tensor_copy(pk[:, 1, :], i2f[:])
                nc.vector.tensor_copy(pk[:, 2, :], g1[:])
                nc.vector.tensor_copy(pk[:, 3, :], g2[:])
                nc.sync.dma_start(pay_in.rearrange("(a p tt) -> p a tt", a=4, p=128), pk[:])

                # gate_proj weights: 2MB links, each gated on the previous via
                # a tiny DVE copy; the chain head hangs off the payload pack so
                # the stream starts right after the (critical) payload write
                # and never monopolizes the DMA engines
                wgt_v = wgt_d.rearrange("(o q) f -> q o f", q=128)
                for h in range(4):
                    src = pk[0:1, 3, 0:2] if h == 0 else wgt_sb[0:1, 2 * (h - 1), 0:2]
                    nc.vector.tensor_copy(wgt_sb[0:1, 2 * h, 0:2], src)
                    nc.sync.dma_start(
                        wgt_sb[:, 2 * h : 2 * (h + 1), :], wgt_v[:, 2 * h : 2 * (h + 1), :]
                    )

                nc.gpsimd.collective_compute(
                    "AllGather", OP.bypass,
                    replica_groups=[list(range(NC))],
                    ins=[pay_in[:].opt()], outs=[pay_all[:].opt()],
                )

                nc.scalar.dma_start(pay_tab[T : T + 1, :], zeros64[0:1, 0:4])

                # reread all 4 arrays into global routing layout [128, 64]
                # (t = 64p + i). NOTE: a merged/rearranged form is NOT safe
                # here — splitting the SBUF partition dim (e.g. "(r p16) i ->
                # r p16 i") silently drops partition semantics (partition_size
                # becomes r) and writes garbage on hardware.
                rt = pr.tile([128, 4, 64], F32)
                pay_view = pay_all.rearrange("(r a p16 i) -> r p16 a i", r=NC, a=4, p16=16)
                for r in range(NC):
                    nc.sync.dma_start(rt[16 * r : 16 * r + 16, :, :], pay_view[r])
                i1r, i2r = rt[:, 0, :], rt[:, 1, :]
                g1r, g2r = rt[:, 2, :], rt[:, 3, :]



                # =============== SLOT ASSIGNMENT (replicated) ===============
                # (the real compiler only allows generic vector ops on DVE,
                # so both choice chains share it)
                v1, v2 = nc.vector, nc.vector
                m1 = pr.tile([128, E, 64], F32)
                m2 = pr.tile([128, E, 64], F32)
                sc1 = pr.tile([128, E, 64], F32)
                sc2 = pr.tile([128, E, 64], F32)
                v1.tensor_tensor(
                    out=m1[:], in0=i1r[:, None, :].to_broadcast([128, E, 64]),
                    in1=eidx[:, :, None].to_broadcast([128, E, 64]), op=OP.is_equal,
                )
                v2.tensor_tensor(
                    out=m2[:], in0=i2r[:, None, :].to_broadcast([128, E, 64]),
                    in1=eidx[:, :, None].to_broadcast([128, E, 64]), op=OP.is_equal,
                )
                for e in range(E):
                    v1.tensor_tensor_scan(sc1[:, e, :], m1[:, e, :], zeros64[:], 0.0, op0=OP.add, op1=OP.add)
                    v2.tensor_tensor_scan(sc2[:, e, :], m2[:, e, :], zeros64[:], 0.0, op0=OP.add, op1=OP.add)
                tot1 = pr.tile([128, E], F32)
                tot2 = pr.tile([128, E], F32)
                v1.tensor_copy(tot1[:], sc1[:, :, 63])
                v2.tensor_copy(tot2[:], sc2[:, :, 63])

                of1_ps = pss.tile([128, E], F32, space="PSUM", tag="ps_small")
                nc.tensor.matmul(of1_ps[:], lhsT=sl[:], rhs=tot1[:], start=True, stop=True)
                of1 = pr.tile([128, E], F32)
                nc.vector.tensor_scalar_add(of1[:], of1_ps[:], -1.0)
                of2_ps = pss.tile([128, E], F32, space="PSUM", tag="ps_small")
                nc.tensor.matmul(of2_ps[:], lhsT=sl[:], rhs=tot2[:], start=True, stop=False)
                nc.tensor.matmul(of2_ps[:], lhsT=ones128[:], rhs=tot1[:], start=False, stop=True)
                of2 = pr.tile([128, E], F32)
                nc.vector.tensor_scalar_add(of2[:], of2_ps[:], -1.0)

                def loc_s(vv, sc, m, of, tag):
                    tmp = pr.tile([128, E, 64], F32, tag=f"loc_tmp{tag}")
                    for e in range(E):
                        vv.scalar_tensor_tensor(
                            out=tmp[:, e, :], in0=sc[:, e, :], scalar=of[:, e : e + 1],
                            in1=m[:, e, :], op0=OP.add, op1=OP.mult,
                        )
                    cur, w = tmp, E
                    while w > 1:
                        nxt = pr.tile([128, w // 2, 64], F32, tag=f"loc_s{tag}{w}")
                        vv.tensor_tensor(out=nxt[:], in0=cur[:, : w // 2, :], in1=cur[:, w // 2 :, :], op=OP.add)
                        cur, w = nxt, w // 2
                    return cur  # [128, 1, 64]

                l1s = loc_s(v1, sc1, m1, of1, "a")[:, 0, :]
                l2s = loc_s(v2, sc2, m2, of2, "b")[:, 0, :]

                def keep_f(vv, ls, ir, tag):
                    kp = pr.tile([128, 64], F32, tag=f"kp{tag}")
                    vv.tensor_scalar(out=kp[:], in0=ls, scalar1=float(C), scalar2=None, op0=OP.is_lt)
                    lc = pr.tile([128, 64], F32, tag=f"lc{tag}")
                    vv.tensor_scalar(out=lc[:], in0=ls, scalar1=float(C - 1), scalar2=None, op0=OP.min)
                    f = pr.tile([128, 64], F32, tag=f"f{tag}")
                    vv.scalar_tensor_tensor(out=f[:], in0=ir, scalar=float(C), in1=lc[:], op0=OP.mult, op1=OP.add)
                    return f, kp

                f1, kp1 = keep_f(v1, l1s, i1r, "a")
                f2, kp2 = keep_f(v2, l2s, i2r, "b")

                # payload table rows t = 64p + i: (f1, f2, g1, g2)
                pt_sb = pr.tile([128, 64, 4], F32)
                nc.vector.tensor_copy(pt_sb[:, :, 0], f1[:])
                nc.vector.tensor_copy(pt_sb[:, :, 1], f2[:])
                nc.vector.tensor_copy(pt_sb[:, :, 2], g1r)
                nc.vector.tensor_copy(pt_sb[:, :, 3], g2r)
                nc.sync.dma_start(
                    pay_tab[0:T, :].rearrange("(p i) c -> p i c", p=128), pt_sb[:]
                )

                # ====== SLOT -> TOKEN MAP (local_scatter + merge + diagonal) ======
                tp1 = pr.tile([128, 64], F32)
                nc.vector.tensor_scalar_add(tp1[:], tif[:], 1.0)   # token id + 1

                def slot_halves(vv, ls, ir, kp, tag):
                    # sel = (expert == cid) && kept; slot+1 where selected else 0
                    isc = pr.tile([128, 64], F32, tag=f"isc{tag}")
                    vv.tensor_tensor(out=isc[:], in0=ir, in1=cid[:, 0:1].to_broadcast([128, 64]), op=OP.is_equal)
                    sel = pr.tile([128, 64], F32, tag=f"sel{tag}")
                    vv.tensor_tensor(out=sel[:], in0=isc[:], in1=kp[:], op=OP.mult)
                    sp1 = pr.tile([128, 64], F32, tag=f"sp1{tag}")  # sel ? slot+1 : 0
                    vv.tensor_scalar_add(sp1[:], ls, 1.0)
                    vv.tensor_tensor(out=sp1[:], in0=sp1[:], in1=sel[:], op=OP.mult)
                    # lo half: slot in [0, 1024): idx = slot, else -1
                    mlo = pr.tile([128, 64], F32, tag=f"mlo{tag}")
                    vv.tensor_scalar(out=mlo[:], in0=sp1[:], scalar1=1024.0, scalar2=None, op0=OP.is_le)
                    vv.tensor_tensor(out=mlo[:], in0=mlo[:], in1=sel[:], op=OP.mult)
                    ilo = pr.tile([128, 64], F32, tag=f"ilo{tag}")
                    vv.tensor_tensor(out=ilo[:], in0=mlo[:], in1=sp1[:], op=OP.mult)
                    vv.tensor_scalar_add(ilo[:], ilo[:], -1.0)
                    # hi half: slot in [1024, 2048): idx = slot - 1024, else -1
                    mhi = pr.tile([128, 64], F32, tag=f"mhi{tag}")
                    vv.tensor_scalar(out=mhi[:], in0=sp1[:], scalar1=1024.0, scalar2=None, op0=OP.is_gt)
                    ihi = pr.tile([128, 64], F32, tag=f"ihi{tag}")
                    vv.tensor_scalar_add(ihi[:], sp1[:], -1024.0)
                    vv.tensor_tensor(out=ihi[:], in0=ihi[:], in1=mhi[:], op=OP.mult)
                    vv.tensor_scalar_add(ihi[:], ihi[:], -1.0)
                    return ilo, ihi

                i1lo, i1hi = slot_halves(v1, l1s, i1r, kp1, "a")
                i2lo, i2hi = slot_halves(v2, l2s, i2r, kp2, "b")

                data128 = pr.tile([128, 128], I16)
                v1.tensor_copy(data128[:, :64], tp1[:])
                v2.tensor_copy(data128[:, 64:], tp1[:])
                idxlo = pr.tile([128, 128], I16)
                v1.tensor_copy(idxlo[:, :64], i1lo[:])
                v2.tensor_copy(idxlo[:, 64:], i2lo[:])
                idxhi = pr.tile([128, 128], I16)
                v1.tensor_copy(idxhi[:, :64], i1hi[:])
                v2.tensor_copy(idxhi[:, 64:], i2hi[:])

                dst_lo = pr.tile([128, 1024], I16)
                nc.gpsimd.local_scatter(dst_lo[:], data128[:], idxlo[:], channels=128, num_elems=1024, num_idxs=128)
                dst_hi = pr.tile([128, 1024], I16)
                nc.gpsimd.local_scatter(dst_hi[:], data128[:], idxhi[:], channels=128, num_elems=1024, num_idxs=128)

                # merge across partitions: each slot column has at most one
                # nonzero writer, so a gpsimd partition all-reduce (max)
                # replicates the slot->token map onto every partition
                merged = pr.tile([128, 2, 1024], F32)  # map+1 on all partitions
                nc.gpsimd.partition_all_reduce(
                    merged[:, 0, :], dst_lo[:], channels=128, reduce_op=bass_isa.ReduceOp.max
                )
                nc.gpsimd.partition_all_reduce(
                    merged[:, 1, :], dst_hi[:], channels=128, reduce_op=bass_isa.ReduceOp.max
                )

                # diagonal extraction: tokraw[p, k] = merged-flat[128k + p]
                tokraw = pr.tile([128, C // 128], F32)
                scratch = pr.tile([128, 128], F32, tag="diag_scr")
                mview = merged[:].rearrange("p a b -> p (a b)")
                scratch2 = pr.tile([128, 128], F32, tag="diag_scr2")
                iszero = pr.tile([128, C // 128], F32)

                def diag_cols(k0, k1):
                    # extract columns [k0,k1), sanitize (0 -> T+1; v -> v-1),
                    # and publish them to tokc so dependent gathers can start
                    for k in range(k0, k1):
                        vv, scr = (v1, scratch) if k % 2 == 0 else (v2, scratch2)
                        vv.scalar_tensor_tensor(
                            out=scr[:], in0=mview[:, 128 * k : 128 * (k + 1)], scalar=0.0,
                            in1=ident[:], op0=OP.add, op1=OP.mult,
                            accum_out=tokraw[:, k : k + 1],
                        )
                    nc.vector.tensor_scalar(out=iszero[:, k0:k1], in0=tokraw[:, k0:k1], scalar1=0.0, scalar2=None, op0=OP.is_equal)
                    nc.vector.scalar_tensor_tensor(
                        out=tokraw[:, k0:k1], in0=iszero[:, k0:k1], scalar=float(T + 1),
                        in1=tokraw[:, k0:k1], op0=OP.mult, op1=OP.add,
                    )
                    nc.vector.tensor_scalar_add(tokraw[:, k0:k1], tokraw[:, k0:k1], -1.0)
                    nc.vector.tensor_copy(tokc[:, k0:k1], tokraw[:, k0:k1])

                # cb0's dispatch gathers need only the first 4 columns: emit
                # them first so the FFN pipeline starts while the rest extract
                diag_cols(0, CBLK // 128)
                diag_cols(CBLK // 128, C // 128)

                # down_proj weights: 2MB links chained like wgt, head gated on
                # the slot map; needed only by cb0's mm2 (~140us)
                wdn_v = wdn_d.rearrange("(o q) d -> q o d", q=128)
                for h in range(4):
                    src = tokraw[0:1, 0:2] if h == 0 else wdn_sb[0:1, 8 * (h - 1), 0:2]
                    nc.vector.tensor_copy(wdn_sb[0:1, 8 * h, 0:2], src)
                    nc.sync.dma_start(
                        wdn_sb[:, 8 * h : 8 * (h + 1), :], wdn_v[:, 8 * h : 8 * (h + 1), :]
                    )

            _route_cm.__exit__(None, None, None)

            # =============== EXPERT FFN (bf16) ===============
            with (
                tc.tile_pool(name="ffn", bufs=1) as pf,
                tc.tile_pool(name="ffn_db", bufs=2) as pfd,
                tc.tile_pool(name="ffn_dr", bufs=4) as pdr,
                tc.tile_pool(name="psum_mm", bufs=2, space="PSUM") as psm,
            ):
                def emit_dispatch(cb):
                    # gather 4 x 128 slot rows and PE-transpose into dispT.
                    # (xbar DMA transposes were tried here — semantically
                    # correct and cheap on paper, but they queue behind the
                    # weight/zero chains on the single FIFO DMA device and
                    # lose 11-26us; PE transposes overlap cleanly.)
                    KT = CBLK // 128
                    dispT = pfd.tile([128, D // 128, CBLK], BF16, tag="dispT")
                    for kt in range(KT):
                        k = KT * cb + kt
                        drow = pdr.tile([128, D], BF16, tag="drow")
                        nc.gpsimd.indirect_dma_start(
                            out=drow[:], out_offset=None, in_=xb[:],
                            in_offset=bass.IndirectOffsetOnAxis(ap=tokc[:, k : k + 1], axis=0),
                        )
                        for dt in range(D // 128):
                            tr_ps = psm.tile([128, 128], BF16, space="PSUM", tag="ps_tr")
                            nc.tensor.transpose(tr_ps[:], drow[:, 128 * dt : 128 * (dt + 1)], ident_bf[:])
                            nc.vector.tensor_copy(dispT[:, dt, 128 * kt : 128 * (kt + 1)], tr_ps[:])
                    return dispT

                next_dispT = emit_dispatch(0)
                for cb in range(NCB):
                    dispT = next_dispT

                    if cb == 0:
                        # slot gates: gather payload rows by slot owner, then
                        # gate = (f1==slot)*g1 + (f2==slot)*g2. Emitted after
                        # cb0's dispatch gathers so they don't delay the FFN
                        # start on the (in-order) gpsimd queue; results are
                        # only needed by cb0's mm2 scale, ~100us later.
                        pg = pf.tile([128, C // 128, 4], F32, tag="pg")
                        for k in range(C // 128):
                            nc.gpsimd.indirect_dma_start(
                                out=pg[:, k, :], out_offset=None, in_=pay_tab[:],
                                in_offset=bass.IndirectOffsetOnAxis(ap=tokc[:, k : k + 1], axis=0),
                            )
                        is1g = pf.tile([128, C // 128], F32, tag="is1g")
                        nc.vector.tensor_tensor(out=is1g[:], in0=pg[:, :, 0], in1=slotid[:], op=OP.is_equal)
                        is2g = pf.tile([128, C // 128], F32, tag="is2g")
                        nc.vector.tensor_tensor(out=is2g[:], in0=pg[:, :, 1], in1=slotid[:], op=OP.is_equal)
                        ga = pf.tile([128, C // 128], F32, tag="ga")
                        nc.vector.tensor_tensor(out=ga[:], in0=is1g[:], in1=pg[:, :, 2], op=OP.mult)
                        gb = pf.tile([128, C // 128], F32, tag="gb")
                        nc.vector.tensor_tensor(out=gb[:], in0=is2g[:], in1=pg[:, :, 3], op=OP.mult)
                        nc.vector.tensor_tensor(out=slotg[:], in0=ga[:], in1=gb[:], op=OP.add)

                    hT = pf.tile([128, F // 128, CBLK], BF16, tag="hT")
                    for ft in range(F // 128):
                        ps1 = psm.tile([128, CBLK], F32, space="PSUM", tag="ps1")
                        for kd in range(D // 128):
                            nc.tensor.matmul(
                                ps1[:],
                                lhsT=wgt_sb[:, kd, 128 * ft : 128 * ft + 128],
                                rhs=dispT[:, kd, :],
                                start=(kd == 0), stop=(kd == D // 128 - 1),
                            )
                        nc.scalar.activation(hT[:, ft, :], ps1[:], AF.Gelu)

                    # prefetch the next block's dispatch AFTER this block's mm1
                    # emission: Tile's in-order PE queue then runs those
                    # transposes only when their gathers are long done, instead
                    # of idling PE mid-mm1 waiting for them
                    if cb + 1 < NCB:
                        next_dispT = emit_dispatch(cb + 1)

                    # mm2 with swapped operands: eo[c, d] = hT.T @ w_down -> row-major out
                    # PSUM->SBUF copy applies the slot gate (per-partition scalar)
                    eo_sb = pf.tile([128, CBLK // 128, D], BF16, tag="eo_sb")
                    for ct in range(CBLK // 128):
                        k = (CBLK // 128) * cb + ct
                        for dc in range(D // 512):
                            ps2 = psm.tile([128, 512], F32, space="PSUM", tag="ps2")
                            for ft in range(F // 128):
                                nc.tensor.matmul(
                                    ps2[:],
                                    lhsT=hT[:, ft, 128 * ct : 128 * ct + 128],
                                    rhs=wdn_sb[:, ft, 512 * dc : 512 * dc + 512],
                                    start=(ft == 0), stop=(ft == F // 128 - 1),
                                )
                            nc.vector.tensor_scalar_mul(
                                eo_sb[:, ct, 512 * dc : 512 * dc + 512], ps2[:],
                                slotg[:, k : k + 1],
                            )
                        # scatter scaled rows into token space (trash row T for
                        # empty slots; their eo is exactly 0 anyway). The static
                        # out AP is a 128-row window (offset must be 0): the
                        # actual rows come from the dynamic offsets, but the
                        # cost model (and descriptor count) key on the static
                        # AP, which must not span the whole 16.8MB tensor.
                        nc.gpsimd.indirect_dma_start(
                            out=part_d[0:T, :].rearrange("(a b) d -> a (b d)", b=64)[:, 0:D],
                            out_offset=bass.IndirectOffsetOnAxis(ap=tokc[:, k : k + 1], axis=0),
                            in_=eo_sb[:, ct, :], in_offset=None,
                        )

            # =============== COMBINE: ReduceScatter over token space ===============
            # (collectives may not read or write IO tensors: internal in/out,
            # then a small DMA moves the reduced shard to y)
            nc.gpsimd.collective_compute(
                "ReduceScatter", OP.add,
                replica_groups=[list(range(NC))],
                ins=[part_d[0:T, :].opt()], outs=[rs_out[:].opt()],
            )
            nc.sync.dma_start(y_d[:], rs_out[:])

    nc.compile()
    return nc


_PROGRAM = None


def _get_program():
    global _PROGRAM
    if _PROGRAM is None:
        _PROGRAM = _build_program()
    return _PROGRAM


def host_constants():
    p = np.arange(128)
    return {
        "ident": np.eye(128, dtype=np.float32),
        "slmat": (np.arange(128)[None, :] > p[:, None]).astype(np.float32),
        "tidx": (64 * p[:, None] + np.arange(64)[None, :]).astype(np.float32),
        "eidx": np.tile(np.arange(E, dtype=np.float32), (128, 1)),
    }


def _make_in_maps(x, wg, w_gate, w_down):
    x = np.asarray(x, np.float32)
    wg_np = np.asarray(wg, np.float32)
    w_gate_np = np.asarray(w_gate, np.float32)
    w_down_np = np.asarray(w_down, np.float32)

    tokens = x.reshape(T, D)
    xb = np.zeros((T + 1, D), ml_dtypes.bfloat16)
    xb[:T] = tokens.astype(ml_dtypes.bfloat16)

    # shard m holds tokens [SH*m, SH*(m+1)); its xT columns are permuted so that
    # matmul tile position j = 128*tt + p corresponds to local token 8*p + tt,
    # making the routing payload DMA contiguous.
    j = np.arange(SH)
    perm = 8 * (j % 128) + j // 128  # local token index at column position j
    consts = host_constants()
    p = np.arange(128)
    kk = np.arange(C // 128)

    in_maps = []
    for m in range(NC):
        shard = tokens[SH * m : SH * (m + 1)]
        xT_sh = np.ascontiguousarray(shard[perm].T)
        in_maps.append({
            "xT_sh": xT_sh,
            "xb": xb,
            "wg": wg_np,
            "wgt": np.ascontiguousarray(w_gate_np[m].astype(ml_dtypes.bfloat16)),
            "wdn": np.ascontiguousarray(w_down_np[m].astype(ml_dtypes.bfloat16)),
            "cid": np.full((128, 1), float(m), np.float32),
            "slotid": (m * C + 128 * kk[None, :] + p[:, None]).astype(np.float32),
            "zsrc": np.zeros((SH, D), ml_dtypes.bfloat16),
            **consts,
        })
    return in_maps


def kernel(x, wg, w_gate, w_down, _trace=False):
    global LAST_RESULT
    x = np.asarray(x, np.float32)
    in_maps = _make_in_maps(x, wg, w_gate, w_down)

    nc = _get_program()
    res = run_bass_kernel_spmd(nc, in_maps, core_ids=list(range(NC)), trace=_trace)
    LAST_RESULT = res
    out = np.concatenate([res.results[m]["y"] for m in range(NC)], axis=0)
    return out.reshape(B, S, D).astype(x.dtype)


def bench(x, wg, w_gate, w_down, iters=6):
    """Measure per-execution wall time with device-resident inputs.

    Returns (output, per_call_seconds_list) where each call gets freshly
    zeroed (donated) output buffers, matching run_bass_via_pjrt semantics.
    """
    import time
    import jax
    from jax.sharding import Mesh, PartitionSpec, NamedSharding
    from jax.experimental.shard_map import shard_map
    import concourse.mybir as _mybir
    from concourse.bass2jax import _bass_exec_p, install_neuronx_cc_hook, partition_id_tensor

    install_neuronx_cc_hook()
    nc = _get_program()

    x = np.asarray(x, np.float32)
    in_maps = _make_in_maps(x, wg, w_gate, w_down)

    in_names, out_names, out_avals, zero_outs = [], [], [], []
    for alloc in nc.m.functions[0].allocations:
        if not isinstance(alloc, _mybir.MemoryLocationSet):
            continue
        name = alloc.memorylocations[0].name
        if alloc.kind == "ExternalInput":
            if nc.partition_id_tensor is None or name != nc.partition_id_tensor.name:
                in_names.append(name)
        elif alloc.kind == "ExternalOutput":
            shape = tuple(alloc.tensor_shape)
            dtype = _mybir.dt.np(alloc.dtype)
            out_names.append(name)
            out_avals.append(jax.core.ShapedArray(shape, dtype))
            zero_outs.append(np.zeros(shape, dtype))
    n_params = len(in_names)
    all_in_names = in_names + out_names
    if nc.partition_id_tensor is not None:
        all_in_names = all_in_names + [nc.partition_id_tensor.name]

    def _body(*args):
        operands = list(args)
        if nc.partition_id_tensor is not None:
            operands.append(partition_id_tensor())
        outs = _bass_exec_p.bind(
            *operands,
            out_avals=tuple(out_avals),
            in_names=tuple(all_in_names),
            out_names=tuple(out_names),
            lowering_input_output_aliases=(),
            sim_require_finite=True,
            sim_require_nnan=True,
            nc=nc,
        )
        return tuple(outs)

    devices = jax.devices()[:NC]
    mesh = Mesh(np.asarray(devices), ("core",))
    nsh = NamedSharding(mesh, PartitionSpec("core"))
    n_outs = len(out_avals)
    donate = tuple(range(n_params, n_params + n_outs))
    sharded = jax.jit(
        shard_map(_body, mesh=mesh, in_specs=(PartitionSpec("core"),) * (n_params + n_outs),
                  out_specs=(PartitionSpec("core"),) * n_outs, check_rep=False),
        donate_argnums=donate, keep_unused=True,
    )

    concat_in = [
        jax.device_put(np.concatenate([np.asarray(in_maps[c][nm]) for c in range(NC)], axis=0), nsh)
        for nm in in_names
    ]
    zero_sets = [
        [jax.device_put(np.zeros((NC * z.shape[0], *z.shape[1:]), z.dtype), nsh) for z in zero_outs]
        for _ in range(iters + 1)
    ]

    out = sharded(*concat_in, *zero_sets[0])  # warmup + compile
    jax.block_until_ready(out)
    times = []
    for it in range(iters):
        t0 = time.perf_counter()
        out = sharded(*concat_in, *zero_sets[it + 1])
        jax.block_until_ready(out)
        times.append(time.perf_counter() - t0)

    outs = {
        nm: np.asarray(out[i]).reshape(NC, *out_avals[i].shape) for i, nm in enumerate(out_names)
    }
    y = np.concatenate([outs["y"][m] for m in range(NC)], axis=0).reshape(B, S, D).astype(x.dtype)
    return y, times


# revision 92
# speedup vs baseline: 1.0423x; 1.0009x over previous
"""MoE layer (GShard top-2 routing + per-expert FFN) on 8 Trainium2 NeuronCores.

Strategy (expert parallelism, ReduceScatter combine):
  - Router matmul (fp32, exact) is token-sharded: each core computes logits for
    its 1024-token shard, then an AllGather shares per-token routing scalars
    (idx1, idx2, g1, g2) with all cores.
  - Every core replicates the (cheap) global slot-assignment math: per-expert
    inclusive scans along the free dim + a triangular-matmul partition prefix
    give each token its capacity slot exactly as the reference's cumsum does.
  - Each core owns ONE expert. The slot->token map is built with local_scatter
    (per-partition scatter of token ids by slot), merged across partitions
    with a gpsimd partition all-reduce (each slot column has one writer), and
    read out column-major via a diagonal extraction (first 4 columns early so
    cb0's dispatch gathers start while the rest extract).
  - Dispatch: 16 indirect row gathers from x (bf16) + PE transposes give the
    [d, slot] layout; FFN in bf16 with fp32 accumulation:
    hT = gelu(w_gate^T @ dispT), eo = hT^T @ w_down (row-major out).
  - Combine via ReduceScatter: a [T+1,4] payload table (f1,f2,g1,g2 per token)
    is written to DRAM and gathered by the slot->token map, giving each slot
    its owner's gate. mm2's PSUM->SBUF copy scales eo rows by that gate, and
    the scaled rows are indirect-scattered into a [T+1,D] bf16 token-space
    partial buffer ("part", zero-filled on device early in the run; the
    collective verifier forbids IO tensors, so it must stay internal). A
    single bf16 ReduceScatter(add) over part[0:T] then yields each core's
    final output shard directly (tokens are shard-ordered), bounced to y
    (bf16) and cast to fp32 on the host.

  Scheduling notes (the TimelineSim cost model serializes all DMA on one
  device, FIFO by acquire time, and Tile schedules by dependency, not
  program order):
  - Big loads (weights, zero-fill) run as single-in-flight chains: each link
    is gated on the previous via a tiny DVE copy (weights) or a RAW
    self-copy (zero chunks), so routing-critical DMAs (payload write, AG
    rereads, dispatch gathers) never wait more than one ~3-6us link.
  - Indirect scatters claim a strided static window (rows 0,64,...,8128) of
    "part": cost is charged on the static AP (256KB, not 16.8MB), while the
    window still overlaps every zero chunk so Tile orders all scatters after
    the zero fill. Do NOT "slice" SBUF partition dims via rearrange in DMA
    APs (e.g. "(r p16) i -> r p16 i") — partition_size silently becomes r
    and the transfer writes garbage on hardware.
"""

import sys

if "/opt/trn_rl_repo" not in sys.path:
    sys.path.insert(0, "/opt/trn_rl_repo")

import numpy as np
import ml_dtypes

import concourse.bacc as bacc
import concourse.mybir as mybir
import concourse.tile as tile
from concourse import bass
from concourse import bass_isa
from concourse.bass_utils import run_bass_kernel_spmd

BF16 = mybir.dt.bfloat16
F32 = mybir.dt.float32
I16 = mybir.dt.int16
I32 = mybir.dt.int32
AF = mybir.ActivationFunctionType
OP = mybir.AluOpType
AX = mybir.AxisListType

B, S, D, E, F = 4, 2048, 1024, 8, 4096
T = B * S            # 8192 tokens
C = 2 * T // E       # 2048 capacity
NC = 8               # cores
SH = T // NC         # 1024 tokens per shard
CBLK = 512           # FFN slot-block
NCB = C // CBLK      # 4 blocks

LAST_RESULT = None   # BassKernelResults of the most recent run (for profiling)


def _build_program():
    nc = bacc.Bacc("TRN2", target_bir_lowering=False, debug=False, num_devices=NC)

    # ---- per-core external inputs ----
    xT_sh = nc.dram_tensor("xT_sh", [D, SH], F32, kind="ExternalInput").ap()
    xb = nc.dram_tensor("xb", [T + 1, D], BF16, kind="ExternalInput").ap()
    wg_d = nc.dram_tensor("wg", [D, E], F32, kind="ExternalInput").ap()
    wgt_d = nc.dram_tensor("wgt", [D, F], BF16, kind="ExternalInput").ap()
    wdn_d = nc.dram_tensor("wdn", [F, D], BF16, kind="ExternalInput").ap()
    cid_d = nc.dram_tensor("cid", [128, 1], F32, kind="ExternalInput").ap()
    slotid_d = nc.dram_tensor("slotid", [128, C // 128], F32, kind="ExternalInput").ap()
    # host-generated constants (gpsimd iota/affine_select aren't available)
    ident_d = nc.dram_tensor("ident", [128, 128], F32, kind="ExternalInput").ap()
    slmat_d = nc.dram_tensor("slmat", [128, 128], F32, kind="ExternalInput").ap()
    tidx_d = nc.dram_tensor("tidx", [128, 64], F32, kind="ExternalInput").ap()
    eidx_d = nc.dram_tensor("eidx", [128, E], F32, kind="ExternalInput").ap()
    carrym_d = nc.dram_tensor("carrym", [128, E * 64], F32, kind="ExternalInput").ap()
    y_d = nc.dram_tensor("y", [SH, D], BF16, kind="ExternalOutput").ap()
    # token-space partial output; zero-filled on device early in the run
    # (collectives may not read IO tensors, so this must stay internal)
    part_d = nc.dram_tensor("part", [T + 1, D], BF16).ap()

    zsrc_d = nc.dram_tensor("zsrc", [SH, D], BF16, kind="ExternalInput").ap()

    # ---- internal DRAM ----
    pay_in = nc.dram_tensor("pay_in", [4 * SH], F32).ap()
    pay_all = nc.dram_tensor("pay_all", [NC * 4 * SH], F32, addr_space="Shared").ap()
    pay_tab = nc.dram_tensor("pay_tab", [T + 1, 4], F32).ap()
    rs_out = nc.dram_tensor("rs_out", [SH, D], BF16).ap()

    with tile.TileContext(nc) as tc:
        with (
            tc.tile_pool(name="persist", bufs=1) as pp,
            tc.tile_pool(name="psum_s", bufs=2, space="PSUM") as pss,
        ):
            # route pool is opened here (before the persist consts, so xT's
            # DMA is emitted first) and closed explicitly before the FFN to
            # free its SBUF
            _route_cm = tc.tile_pool(name="route", bufs=1)
            pr = _route_cm.__enter__()

            # xT is the head of the critical path: emit it before everything
            # else so it gets the first DMA slot
            xT_sb = pr.tile([128, D // 128, SH], F32)
            nc.sync.dma_start(xT_sb[:], xT_sh.rearrange("(o q) t -> q o t", q=128))
            wg_sb = pr.tile([128, D // 128, E], F32)
            nc.sync.dma_start(wg_sb[:], wg_d.rearrange("(o q) e -> q o e", q=128))

            # zero-fill the token-space partial buffer. The 1MB chunks chain
            # off each other (RAW on the previous chunk), so at most one is in
            # flight and later critical DMAs (payload write, AG rereads,
            # dispatch gathers) wait at most ~3us for the DMA engines. Must
            # complete before the first eo scatter (~150us).
            ZC = 512
            nc.scalar.dma_start(part_d[0:ZC, :], zsrc_d[0:ZC, :])
            for zc in range(1, T // ZC):
                nc.scalar.dma_start(
                    part_d[ZC * zc : ZC * (zc + 1), :],
                    part_d[ZC * (zc - 1) : ZC * zc, :],
                )

            ident = pp.tile([128, 128], F32)
            nc.sync.dma_start(ident[:], ident_d[:])
            ident_bf = pp.tile([128, 128], BF16)
            nc.vector.tensor_copy(ident_bf[:], ident[:])
            cid = pp.tile([128, 1], F32)
            nc.sync.dma_start(cid[:], cid_d[:])
            slotid = pp.tile([128, C // 128], F32)
            nc.sync.dma_start(slotid[:], slotid_d[:])
            zeros64 = pp.tile([128, 64], F32)
            nc.vector.memset(zeros64[:], 0.0)
            ones128 = pp.tile([128, 128], F32)
            nc.vector.memset(ones128[:], 1.0)

            # resident expert weights (bf16)
            wgt_sb = pp.tile([128, D // 128, F], BF16)
            wdn_sb = pp.tile([128, F // 128, D], BF16)



            # persistent routing products
            tokc = pp.tile([128, C // 128], I32)    # dispatch: slot->token, col-major
            slotg = pp.tile([128, C // 128], F32)   # gate per slot, col-major

            # =============== ROUTER (token shard, fp32) ===============
            if True:
                sl = pr.tile([128, 128], F32)
                nc.sync.dma_start(sl[:], slmat_d[:])
                tif = pr.tile([128, 64], F32)
                nc.sync.dma_start(tif[:], tidx_d[:])
                eidx = pr.tile([128, E], F32)
                nc.sync.dma_start(eidx[:], eidx_d[:])
                carrym = pr.tile([128, E * 64], F32)
                nc.sync.dma_start(carrym[:], carrym_d[:])



                lg = pr.tile([128, 8, E], F32)  # logits, token pos j = 128*tt + p
                for tt in range(8):
                    ps = pss.tile([128, E], F32, space="PSUM", tag="ps_small")
                    for kd in range(8):
                        nc.tensor.matmul(
                            ps[:],
                            lhsT=xT_sb[:, kd, 128 * tt : 128 * tt + 128],
                            rhs=wg_sb[:, kd, :],
                            start=(kd == 0),
                            stop=(kd == 7),
                        )
                    nc.vector.tensor_copy(lg[:, tt, :], ps[:])

                m1x = pr.tile([128, 8], F32)
                nc.vector.tensor_reduce(m1x[:], lg[:], AX.X, OP.max)


                is1 = pr.tile([128, 8, E], F32)
                nc.vector.tensor_tensor(
                    out=is1[:], in0=lg[:], in1=m1x[:, :, None].to_broadcast([128, 8, E]),
                    op=OP.is_equal,
                )
                l2 = pr.tile([128, 8, E], F32)
                nc.vector.scalar_tensor_tensor(
                    out=l2[:], in0=is1[:], scalar=-1e30, in1=lg[:], op0=OP.mult, op1=OP.add,
                )
                m2x = pr.tile([128, 8], F32)
                nc.vector.tensor_reduce(m2x[:], l2[:], AX.X, OP.max)
                is2 = pr.tile([128, 8, E], F32)
                nc.vector.tensor_tensor(
                    out=is2[:], in0=l2[:], in1=m2x[:, :, None].to_broadcast([128, 8, E]),
                    op=OP.is_equal,
                )

                # argmax index = sum(mask * eidx) along E
                i1f = pr.tile([128, 8], F32)
                sc1a = pr.tile([128, 8, E], F32, tag="am_scr_a")
                nc.vector.tensor_tensor(
                    out=sc1a[:], in0=is1[:], in1=eidx[:, None, :].to_broadcast([128, 8, E]),
                    op=OP.mult,
                )
                nc.vector.tensor_reduce(i1f[:], sc1a[:], AX.X, OP.add)
                i2f = pr.tile([128, 8], F32)
                sc2a = pr.tile([128, 8, E], F32, tag="am_scr_b")
                nc.vector.tensor_tensor(
                    out=sc2a[:], in0=is2[:], in1=eidx[:, None, :].to_broadcast([128, 8, E]),
                    op=OP.mult,
                )
                nc.vector.tensor_reduce(i2f[:], sc2a[:], AX.X, OP.add)

                # top-2 softmax gates: g1 = 1/(1+exp(m2-m1)), g2 = 1-g1
                dm = pr.tile([128, 8], F32)
                nc.vector.tensor_tensor(out=dm[:], in0=m2x[:], in1=m1x[:], op=OP.subtract)
                e2 = pr.tile([128, 8], F32)
                nc.scalar.activation(e2[:], dm[:], AF.Exp)
                den = pr.tile([128, 8], F32)
                nc.vector.tensor_scalar_add(den[:], e2[:], 1.0)
                g1 = pr.tile([128, 8], F32)
                nc.vector.reciprocal(g1[:], den[:])
                g2 = pr.tile([128, 8], F32)
                nc.vector.tensor_tensor(out=g2[:], in0=e2[:], in1=g1[:], op=OP.mult)

                pk = pr.tile([128, 4, 8], F32)
                nc.vector.tensor_copy(pk[:, 0, :], i1f[:])
                nc.vector.tensor_copy(pk[:, 1, :], i2f[:])
                nc.vector.tensor_copy(pk[:, 2, :], g1[:])
                nc.vector.tensor_copy(pk[:, 3, :], g2[:])
                nc.sync.dma_start(pay_in.rearrange("(a p tt) -> p a tt", a=4, p=128), pk[:])

                # gate_proj weights: 2MB links, each gated on the previous via
                # a tiny DVE copy; the chain head hangs off the payload pack so
                # the stream starts right after the (critical) payload write
                # and never monopolizes the DMA engines
                wgt_v = wgt_d.rearrange("(o q) f -> q o f", q=128)
                for h in range(4):
                    src = pk[0:1, 3, 0:2] if h == 0 else wgt_sb[0:1, 2 * (h - 1), 0:2]
                    nc.vector.tensor_copy(wgt_sb[0:1, 2 * h, 0:2], src)
                    nc.sync.dma_start(
                        wgt_sb[:, 2 * h : 2 * (h + 1), :], wgt_v[:, 2 * h : 2 * (h + 1), :]
                    )

                nc.gpsimd.collective_compute(
                    "AllGather", OP.bypass,
                    replica_groups=[list(range(NC))],
                    ins=[pay_in[:].opt()], outs=[pay_all[:].opt()],
                )

                nc.scalar.dma_start(pay_tab[T : T + 1, :], zeros64[0:1, 0:4])

                # reread all 4 arrays into global routing layout [128, 64]
                # (t = 64p + i). NOTE: a merged/rearranged form is NOT safe
                # here — splitting the SBUF partition dim (e.g. "(r p16) i ->
                # r p16 i") silently drops partition semantics (partition_size
                # becomes r) and writes garbage on hardware.
                rt = pr.tile([128, 4, 64], F32)
                pay_view = pay_all.rearrange("(r a p16 i) -> r p16 a i", r=NC, a=4, p16=16)
                for r in range(NC):
                    nc.sync.dma_start(rt[16 * r : 16 * r + 16, :, :], pay_view[r])
                i1r, i2r = rt[:, 0, :], rt[:, 1, :]
                g1r, g2r = rt[:, 2, :], rt[:, 3, :]



                # =============== SLOT ASSIGNMENT (replicated) ===============
                # (the real compiler only allows generic vector ops on DVE,
                # so both choice chains share it)
                v1, v2 = nc.vector, nc.vector
                m1 = pr.tile([128, E, 64], F32)
                m2 = pr.tile([128, E, 64], F32)
                sc1 = pr.tile([128, E, 64], F32)
                sc2 = pr.tile([128, E, 64], F32)
                v1.tensor_tensor(
                    out=m1[:], in0=i1r[:, None, :].to_broadcast([128, E, 64]),
                    in1=eidx[:, :, None].to_broadcast([128, E, 64]), op=OP.is_equal,
                )
                v2.tensor_tensor(
                    out=m2[:], in0=i2r[:, None, :].to_broadcast([128, E, 64]),
                    in1=eidx[:, :, None].to_broadcast([128, E, 64]), op=OP.is_equal,
                )
                # ONE segmented scan per choice: state = carry*state + m with
                # carry=0 at each expert's first column resets the recurrence
                # at segment boundaries (replaces 8 per-expert scans)
                v1.tensor_tensor_scan(
                    sc1[:].rearrange("p e i -> p (e i)"), carrym[:],
                    m1[:].rearrange("p e i -> p (e i)"), 0.0, op0=OP.mult, op1=OP.add,
                )
                v2.tensor_tensor_scan(
                    sc2[:].rearrange("p e i -> p (e i)"), carrym[:],
                    m2[:].rearrange("p e i -> p (e i)"), 0.0, op0=OP.mult, op1=OP.add,
                )
                tot1 = pr.tile([128, E], F32)
                tot2 = pr.tile([128, E], F32)
                v1.tensor_copy(tot1[:], sc1[:, :, 63])
                v2.tensor_copy(tot2[:], sc2[:, :, 63])

                of1_ps = pss.tile([128, E], F32, space="PSUM", tag="ps_small")
                nc.tensor.matmul(of1_ps[:], lhsT=sl[:], rhs=tot1[:], start=True, stop=True)
                of1 = pr.tile([128, E], F32)
                nc.vector.tensor_scalar_add(of1[:], of1_ps[:], -1.0)
                of2_ps = pss.tile([128, E], F32, space="PSUM", tag="ps_small")
                nc.tensor.matmul(of2_ps[:], lhsT=sl[:], rhs=tot2[:], start=True, stop=False)
                nc.tensor.matmul(of2_ps[:], lhsT=ones128[:], rhs=tot1[:], start=False, stop=True)
                of2 = pr.tile([128, E], F32)
                nc.vector.tensor_scalar_add(of2[:], of2_ps[:], -1.0)

                def loc_s(vv, sc, m, of, tag):
                    # (sc + of) broadcast-added per expert, masked, then a
                    # strided reduce over the expert dim — 3 wide ops instead
                    # of 8 scalar ops + a 3-level tree
                    tmp = pr.tile([128, E, 64], F32, tag=f"loc_tmp{tag}")
                    vv.tensor_tensor(
                        out=tmp[:], in0=sc[:, :, :],
                        in1=of[:, :, None].to_broadcast([128, E, 64]), op=OP.add,
                    )
                    vv.tensor_tensor(out=tmp[:], in0=tmp[:], in1=m[:, :, :], op=OP.mult)
                    ls = pr.tile([128, 64], F32, tag=f"loc_ls{tag}")
                    vv.tensor_reduce(ls[:], tmp[:].rearrange("p e i -> p i e"), AX.X, OP.add)
                    return ls[:]

                l1s = loc_s(v1, sc1, m1, of1, "a")
                l2s = loc_s(v2, sc2, m2, of2, "b")

                def keep_f(vv, ls, ir, tag):
                    kp = pr.tile([128, 64], F32, tag=f"kp{tag}")
                    vv.tensor_scalar(out=kp[:], in0=ls, scalar1=float(C), scalar2=None, op0=OP.is_lt)
                    lc = pr.tile([128, 64], F32, tag=f"lc{tag}")
                    vv.tensor_scalar(out=lc[:], in0=ls, scalar1=float(C - 1), scalar2=None, op0=OP.min)
                    f = pr.tile([128, 64], F32, tag=f"f{tag}")
                    vv.scalar_tensor_tensor(out=f[:], in0=ir, scalar=float(C), in1=lc[:], op0=OP.mult, op1=OP.add)
                    return f, kp

                f1, kp1 = keep_f(v1, l1s, i1r, "a")
                f2, kp2 = keep_f(v2, l2s, i2r, "b")

                # payload table rows t = 64p + i: (f1, f2, g1, g2)
                pt_sb = pr.tile([128, 64, 4], F32)
                nc.vector.tensor_copy(pt_sb[:, :, 0], f1[:])
                nc.vector.tensor_copy(pt_sb[:, :, 1], f2[:])
                nc.vector.tensor_copy(pt_sb[:, :, 2], g1r)
                nc.vector.tensor_copy(pt_sb[:, :, 3], g2r)
                nc.sync.dma_start(
                    pay_tab[0:T, :].rearrange("(p i) c -> p i c", p=128), pt_sb[:]
                )

                # ====== SLOT -> TOKEN MAP (local_scatter + merge + diagonal) ======
                tp1 = pr.tile([128, 64], F32)
                nc.vector.tensor_scalar_add(tp1[:], tif[:], 1.0)   # token id + 1

                def slot_halves(vv, ls, ir, kp, tag):
                    # sel = (expert == cid) && kept; slot+1 where selected else 0
                    isc = pr.tile([128, 64], F32, tag=f"isc{tag}")
                    vv.tensor_tensor(out=isc[:], in0=ir, in1=cid[:, 0:1].to_broadcast([128, 64]), op=OP.is_equal)
                    sel = pr.tile([128, 64], F32, tag=f"sel{tag}")
                    vv.tensor_tensor(out=sel[:], in0=isc[:], in1=kp[:], op=OP.mult)
                    sp1 = pr.tile([128, 64], F32, tag=f"sp1{tag}")  # sel ? slot+1 : 0
                    vv.tensor_scalar_add(sp1[:], ls, 1.0)
                    vv.tensor_tensor(out=sp1[:], in0=sp1[:], in1=sel[:], op=OP.mult)
                    # lo half: slot in [0, 1024): idx = slot, else -1
                    mlo = pr.tile([128, 64], F32, tag=f"mlo{tag}")
                    vv.tensor_scalar(out=mlo[:], in0=sp1[:], scalar1=1024.0, scalar2=None, op0=OP.is_le)
                    vv.tensor_tensor(out=mlo[:], in0=mlo[:], in1=sel[:], op=OP.mult)
                    ilo = pr.tile([128, 64], F32, tag=f"ilo{tag}")
                    vv.tensor_tensor(out=ilo[:], in0=mlo[:], in1=sp1[:], op=OP.mult)
                    vv.tensor_scalar_add(ilo[:], ilo[:], -1.0)
                    # hi half: slot in [1024, 2048): idx = slot - 1024, else -1
                    mhi = pr.tile([128, 64], F32, tag=f"mhi{tag}")
                    vv.tensor_scalar(out=mhi[:], in0=sp1[:], scalar1=1024.0, scalar2=None, op0=OP.is_gt)
                    ihi = pr.tile([128, 64], F32, tag=f"ihi{tag}")
                    vv.tensor_scalar_add(ihi[:], sp1[:], -1024.0)
                    vv.tensor_tensor(out=ihi[:], in0=ihi[:], in1=mhi[:], op=OP.mult)
                    vv.tensor_scalar_add(ihi[:], ihi[:], -1.0)
                    return ilo, ihi

                i1lo, i1hi = slot_halves(v1, l1s, i1r, kp1, "a")
                i2lo, i2hi = slot_halves(v2, l2s, i2r, kp2, "b")

                data128 = pr.tile([128, 128], I16)
                v1.tensor_copy(data128[:, :64], tp1[:])
                v2.tensor_copy(data128[:, 64:], tp1[:])
                idxlo = pr.tile([128, 128], I16)
                v1.tensor_copy(idxlo[:, :64], i1lo[:])
                v2.tensor_copy(idxlo[:, 64:], i2lo[:])
                idxhi = pr.tile([128, 128], I16)
                v1.tensor_copy(idxhi[:, :64], i1hi[:])
                v2.tensor_copy(idxhi[:, 64:], i2hi[:])

                dst_lo = pr.tile([128, 1024], I16)
                nc.gpsimd.local_scatter(dst_lo[:], data128[:], idxlo[:], channels=128, num_elems=1024, num_idxs=128)
                dst_hi = pr.tile([128, 1024], I16)
                nc.gpsimd.local_scatter(dst_hi[:], data128[:], idxhi[:], channels=128, num_elems=1024, num_idxs=128)

                # merge across partitions: each slot column has at most one
                # nonzero writer, so a gpsimd partition all-reduce (max)
                # replicates the slot->token map onto every partition
                merged = pr.tile([128, 2, 1024], F32)  # map+1 on all partitions
                nc.gpsimd.partition_all_reduce(
                    merged[:, 0, :], dst_lo[:], channels=128, reduce_op=bass_isa.ReduceOp.max
                )
                nc.gpsimd.partition_all_reduce(
                    merged[:, 1, :], dst_hi[:], channels=128, reduce_op=bass_isa.ReduceOp.max
                )

                # diagonal extraction: tokraw[p, k] = merged-flat[128k + p]
                tokraw = pr.tile([128, C // 128], F32)
                scratch = pr.tile([128, 128], F32, tag="diag_scr")
                mview = merged[:].rearrange("p a b -> p (a b)")
                scratch2 = pr.tile([128, 128], F32, tag="diag_scr2")
                iszero = pr.tile([128, C // 128], F32)

                def diag_cols(k0, k1):
                    # extract columns [k0,k1), sanitize (0 -> T+1; v -> v-1),
                    # and publish them to tokc so dependent gathers can start
                    for k in range(k0, k1):
                        vv, scr = (v1, scratch) if k % 2 == 0 else (v2, scratch2)
                        vv.scalar_tensor_tensor(
                            out=scr[:], in0=mview[:, 128 * k : 128 * (k + 1)], scalar=0.0,
                            in1=ident[:], op0=OP.add, op1=OP.mult,
                            accum_out=tokraw[:, k : k + 1],
                        )
                    nc.vector.tensor_scalar(out=iszero[:, k0:k1], in0=tokraw[:, k0:k1], scalar1=0.0, scalar2=None, op0=OP.is_equal)
                    nc.vector.scalar_tensor_tensor(
                        out=tokraw[:, k0:k1], in0=iszero[:, k0:k1], scalar=float(T + 1),
                        in1=tokraw[:, k0:k1], op0=OP.mult, op1=OP.add,
                    )
                    nc.vector.tensor_scalar_add(tokraw[:, k0:k1], tokraw[:, k0:k1], -1.0)
                    nc.vector.tensor_copy(tokc[:, k0:k1], tokraw[:, k0:k1])

                # cb0's dispatch gathers need only the first 4 columns: emit
                # them first so the FFN pipeline starts while the rest extract
                diag_cols(0, CBLK // 128)
                diag_cols(CBLK // 128, C // 128)

                # down_proj weights: 2MB links chained like wgt, head gated on
                # the slot map; needed only by cb0's mm2 (~140us)
                wdn_v = wdn_d.rearrange("(o q) d -> q o d", q=128)
                for h in range(4):
                    src = tokraw[0:1, 0:2] if h == 0 else wdn_sb[0:1, 8 * (h - 1), 0:2]
                    nc.vector.tensor_copy(wdn_sb[0:1, 8 * h, 0:2], src)
                    nc.sync.dma_start(
                        wdn_sb[:, 8 * h : 8 * (h + 1), :], wdn_v[:, 8 * h : 8 * (h + 1), :]
                    )

            _route_cm.__exit__(None, None, None)

            # =============== EXPERT FFN (bf16) ===============
            with (
                tc.tile_pool(name="ffn", bufs=1) as pf,
                tc.tile_pool(name="ffn_db", bufs=2) as pfd,
                tc.tile_pool(name="ffn_dr", bufs=4) as pdr,
                tc.tile_pool(name="psum_mm", bufs=2, space="PSUM") as psm,
            ):
                def emit_dispatch(cb):
                    # gather 4 x 128 slot rows and PE-transpose into dispT.
                    # (xbar DMA transposes were tried here — semantically
                    # correct and cheap on paper, but they queue behind the
                    # weight/zero chains on the single FIFO DMA device and
                    # lose 11-26us; PE transposes overlap cleanly.)
                    KT = CBLK // 128
                    dispT = pfd.tile([128, D // 128, CBLK], BF16, tag="dispT")
                    for kt in range(KT):
                        k = KT * cb + kt
                        drow = pdr.tile([128, D], BF16, tag="drow")
                        nc.gpsimd.indirect_dma_start(
                            out=drow[:], out_offset=None, in_=xb[:],
                            in_offset=bass.IndirectOffsetOnAxis(ap=tokc[:, k : k + 1], axis=0),
                        )
                        for dt in range(D // 128):
                            tr_ps = psm.tile([128, 128], BF16, space="PSUM", tag="ps_tr")
                            nc.tensor.transpose(tr_ps[:], drow[:, 128 * dt : 128 * (dt + 1)], ident_bf[:])
                            nc.vector.tensor_copy(dispT[:, dt, 128 * kt : 128 * (kt + 1)], tr_ps[:])
                    return dispT

                next_dispT = emit_dispatch(0)
                for cb in range(NCB):
                    dispT = next_dispT

                    if cb == 0:
                        # slot gates: gather payload rows by slot owner, then
                        # gate = (f1==slot)*g1 + (f2==slot)*g2. Emitted after
                        # cb0's dispatch gathers so they don't delay the FFN
                        # start on the (in-order) gpsimd queue; results are
                        # only needed by cb0's mm2 scale, ~100us later.
                        pg = pf.tile([128, C // 128, 4], F32, tag="pg")
                        for k in range(C // 128):
                            nc.gpsimd.indirect_dma_start(
                                out=pg[:, k, :], out_offset=None, in_=pay_tab[:],
                                in_offset=bass.IndirectOffsetOnAxis(ap=tokc[:, k : k + 1], axis=0),
                            )
                        is1g = pf.tile([128, C // 128], F32, tag="is1g")
                        nc.vector.tensor_tensor(out=is1g[:], in0=pg[:, :, 0], in1=slotid[:], op=OP.is_equal)
                        is2g = pf.tile([128, C // 128], F32, tag="is2g")
                        nc.vector.tensor_tensor(out=is2g[:], in0=pg[:, :, 1], in1=slotid[:], op=OP.is_equal)
                        ga = pf.tile([128, C // 128], F32, tag="ga")
                        nc.vector.tensor_tensor(out=ga[:], in0=is1g[:], in1=pg[:, :, 2], op=OP.mult)
                        gb = pf.tile([128, C // 128], F32, tag="gb")
                        nc.vector.tensor_tensor(out=gb[:], in0=is2g[:], in1=pg[:, :, 3], op=OP.mult)
                        nc.vector.tensor_tensor(out=slotg[:], in0=ga[:], in1=gb[:], op=OP.add)

                    hT = pf.tile([128, F // 128, CBLK], BF16, tag="hT")
                    for ft in range(F // 128):
                        ps1 = psm.tile([128, CBLK], F32, space="PSUM", tag="ps1")
                        for kd in range(D // 128):
                            nc.tensor.matmul(
                                ps1[:],
                                lhsT=wgt_sb[:, kd, 128 * ft : 128 * ft + 128],
                                rhs=dispT[:, kd, :],
                                start=(kd == 0), stop=(kd == D // 128 - 1),
                            )
                        nc.scalar.activation(hT[:, ft, :], ps1[:], AF.Gelu)

                    # prefetch the next block's dispatch AFTER this block's mm1
                    # emission: Tile's in-order PE queue then runs those
                    # transposes only when their gathers are long done, instead
                    # of idling PE mid-mm1 waiting for them
                    if cb + 1 < NCB:
                        next_dispT = emit_dispatch(cb + 1)

                    # mm2 with swapped operands: eo[c, d] = hT.T @ w_down -> row-major out
                    # PSUM->SBUF copy applies the slot gate (per-partition scalar)
                    eo_sb = pf.tile([128, CBLK // 128, D], BF16, tag="eo_sb")
                    for ct in range(CBLK // 128):
                        k = (CBLK // 128) * cb + ct
                        for dc in range(D // 512):
                            ps2 = psm.tile([128, 512], F32, space="PSUM", tag="ps2")
                            for ft in range(F // 128):
                                nc.tensor.matmul(
                                    ps2[:],
                                    lhsT=hT[:, ft, 128 * ct : 128 * ct + 128],
                                    rhs=wdn_sb[:, ft, 512 * dc : 512 * dc + 512],
                                    start=(ft == 0), stop=(ft == F // 128 - 1),
                                )
                            nc.vector.tensor_scalar_mul(
                                eo_sb[:, ct, 512 * dc : 512 * dc + 512], ps2[:],
                                slotg[:, k : k + 1],
                            )
                        # scatter scaled rows into token space (trash row T for
                        # empty slots; their eo is exactly 0 anyway). The static
                        # out AP is a 128-row window (offset must be 0): the
                        # actual rows come from the dynamic offsets, but the
                        # cost model (and descriptor count) key on the static
                        # AP, which must not span the whole 16.8MB tensor.
                        nc.gpsimd.indirect_dma_start(
                            out=part_d[0:T, :].rearrange("(a b) d -> a (b d)", b=64)[:, 0:D],
                            out_offset=bass.IndirectOffsetOnAxis(ap=tokc[:, k : k + 1], axis=0),
                            in_=eo_sb[:, ct, :], in_offset=None,
                        )

            # =============== COMBINE: ReduceScatter over token space ===============
            # (collectives may not read or write IO tensors: internal in/out,
            # then a small DMA moves the reduced shard to y)
            nc.gpsimd.collective_compute(
                "ReduceScatter", OP.add,
                replica_groups=[list(range(NC))],
                ins=[part_d[0:T, :].opt()], outs=[rs_out[:].opt()],
            )
            nc.sync.dma_start(y_d[:], rs_out[:])

    nc.compile()
    return nc


_PROGRAM = None


def _get_program():
    global _PROGRAM
    if _PROGRAM is None:
        _PROGRAM = _build_program()
    return _PROGRAM


def host_constants():
    p = np.arange(128)
    return {
        "ident": np.eye(128, dtype=np.float32),
        "slmat": (np.arange(128)[None, :] > p[:, None]).astype(np.float32),
        "tidx": (64 * p[:, None] + np.arange(64)[None, :]).astype(np.float32),
        "eidx": np.tile(np.arange(E, dtype=np.float32), (128, 1)),
        "carrym": np.tile(np.where(np.arange(E * 64) % 64 == 0, 0.0, 1.0).astype(np.float32), (128, 1)),
    }


def _make_in_maps(x, wg, w_gate, w_down):
    x = np.asarray(x, np.float32)
    wg_np = np.asarray(wg, np.float32)
    w_gate_np = np.asarray(w_gate, np.float32)
    w_down_np = np.asarray(w_down, np.float32)

    tokens = x.reshape(T, D)
    xb = np.zeros((T + 1, D), ml_dtypes.bfloat16)
    xb[:T] = tokens.astype(ml_dtypes.bfloat16)

    # shard m holds tokens [SH*m, SH*(m+1)); its xT columns are permuted so that
    # matmul tile position j = 128*tt + p corresponds to local token 8*p + tt,
    # making the routing payload DMA contiguous.
    j = np.arange(SH)
    perm = 8 * (j % 128) + j // 128  # local token index at column position j
    consts = host_constants()
    p = np.arange(128)
    kk = np.arange(C // 128)

    in_maps = []
    for m in range(NC):
        shard = tokens[SH * m : SH * (m + 1)]
        xT_sh = np.ascontiguousarray(shard[perm].T)
        in_maps.append({
            "xT_sh": xT_sh,
            "xb": xb,
            "wg": wg_np,
            "wgt": np.ascontiguousarray(w_gate_np[m].astype(ml_dtypes.bfloat16)),
            "wdn": np.ascontiguousarray(w_down_np[m].astype(ml_dtypes.bfloat16)),
            "cid": np.full((128, 1), float(m), np.float32),
            "slotid": (m * C + 128 * kk[None, :] + p[:, None]).astype(np.float32),
            "zsrc": np.zeros((SH, D), ml_dtypes.bfloat16),
            **consts,
        })
    return in_maps


def kernel(x, wg, w_gate, w_down, _trace=False):
    global LAST_RESULT
    x = np.asarray(x, np.float32)
    in_maps = _make_in_maps(x, wg, w_gate, w_down)

    nc = _get_program()
    res = run_bass_kernel_spmd(nc, in_maps, core_ids=list(range(NC)), trace=_trace)
    LAST_RESULT = res
    out = np.concatenate([res.results[m]["y"] for m in range(NC)], axis=0)
    return out.reshape(B, S, D).astype(x.dtype)


def bench(x, wg, w_gate, w_down, iters=6):
    """Measure per-execution wall time with device-resident inputs.

    Returns (output, per_call_seconds_list) where each call gets freshly
    zeroed (donated) output buffers, matching run_bass_via_pjrt semantics.
    """
    import time
    import jax
    from jax.sharding import Mesh, PartitionSpec, NamedSharding
    from jax.experimental.shard_map import shard_map
    import concourse.mybir as _mybir
    from concourse.bass2jax import _bass_exec_p, install_neuronx_cc_hook, partition_id_tensor

    install_neuronx_cc_hook()
    nc = _get_program()

    x = np.asarray(x, np.float32)
    in_maps = _make_in_maps(x, wg, w_gate, w_down)

    in_names, out_names, out_avals, zero_outs = [], [], [], []
    for alloc in nc.m.functions[0].allocations:
        if not isinstance(alloc, _mybir.MemoryLocationSet):
            continue
        name = alloc.memorylocations[0].name
        if alloc.kind == "ExternalInput":
            if nc.partition_id_tensor is None or name != nc.partition_id_tensor.name:
                in_names.append(name)
        elif alloc.kind == "ExternalOutput":
            shape = tuple(alloc.tensor_shape)
            dtype = _mybir.dt.np(alloc.dtype)
            out_names.append(name)
            out_avals.append(jax.core.ShapedArray(shape, dtype))
            zero_outs.append(np.zeros(shape, dtype))
    n_params = len(in_names)
    all_in_names = in_names + out_names
    if nc.partition_id_tensor is not None:
        all_in_names = all_in_names + [nc.partition_id_tensor.name]

    def _body(*args):
        operands = list(args)
        if nc.partition_id_tensor is not None:
            operands.append(partition_id_tensor())
        outs = _bass_exec_p.bind(
            *operands,
            out_avals=tuple(out_avals),
            in_names=tuple(all_in_names),
            out_names=tuple(out_names),
            lowering_input_output_aliases=(),
            sim_require_finite=True,
            sim_require_nnan=True,
            nc=nc,
        )
        return tuple(outs)

    devices = jax.devices()[:NC]
    mesh = Mesh(np.asarray(devices), ("core",))
    nsh = NamedSharding(mesh, PartitionSpec("core"))
    n_outs = len(out_avals)
    donate = tuple(range(n_params, n_params + n_outs))
    sharded = jax.jit(
        shard_map(_body, mesh=mesh, in_specs=(PartitionSpec("core"),) * (n_params + n_outs),
                  out_specs=(PartitionSpec("core"),) * n_outs, check_rep=False),
        donate_argnums=donate, keep_unused=True,
    )

    concat_in = [
        jax.device_put(np.concatenate([np.asarray(in_maps[c][nm]) for c in range(NC)], axis=0), nsh)
        for nm in in_names
    ]
    zero_sets = [
        [jax.device_put(np.zeros((NC * z.shape[0], *z.shape[1:]), z.dtype), nsh) for z in zero_outs]
        for _ in range(iters + 1)
    ]

    out = sharded(*concat_in, *zero_sets[0])  # warmup + compile
    jax.block_until_ready(out)
    times = []
    for it in range(iters):
        t0 = time.perf_counter()
        out = sharded(*concat_in, *zero_sets[it + 1])
        jax.block_until_ready(out)
        times.append(time.perf_counter() - t0)

    outs = {
        nm: np.asarray(out[i]).reshape(NC, *out_avals[i].shape) for i, nm in enumerate(out_names)
    }
    y = np.concatenate([outs["y"][m] for m in range(NC)], axis=0).reshape(B, S, D).astype(x.dtype)
    return y, times
